# revision 71
# baseline (speedup 1.0000x reference)
"""Multi-head causal attention (B=4, S=2048, D=1024, 16 heads) on 8 TRN2 cores.

Sharding: core c -> (batch b = c//2, head-group g = c%2). Each core computes
8 heads of one batch element end-to-end (QKV proj, causal softmax attention,
out-proj rows for its head slice). Host sums the two head-group partials per
batch and adds the output bias.

Per-core pipeline (all matmuls contraction-on-partitions, bf16 in / f32 psum):
  QT/KT[dtile] = (x @ w)^T   [128p = 2 heads x 64, S]
  Vones[kb]    = [V | 1]     [128p = k, h, 65]
  attention per (512-wide q-chunk, head-pair); the pair's score matmuls are
  packed into PE row groups via tile_position; two k-blocks share one
  [128,1024] score psum so each exp covers ~1024 cols; PV matmuls trail two
  iterations behind so PE never waits on the exp:
    ST[k,q] = KT.T @ QT; PT = exp(ST/8) bf16; tri-mask on diagonal 128 cols
    ctx[q 128, h, 65] += PT.T @ [V_h | 1]  (PT slice is the lhsT: output has
      q on partitions -> full 128-row PE output, ~half the PV stream cols of
      the [65, q] orientation; col 64 = softmax denominator)
    nq[q, d] = ctx * (1/denominator)  per-partition scale on DVE
    cxtq[d, q] = DMA-xbar-transpose(nq)  per (head-pair, 128-q tile)
  out[seq128, 512] = cxtq.T @ ow, streamed to DRAM per q-chunk; the final
  q-chunk's out-proj borrows the freed score psum banks.
"""

import numpy as np
import ml_dtypes

B, S, D = 4, 2048, 1024
H_TOT = 16
HD = 64
NCORES = 8
GH = 8          # heads per core
GD = GH * HD    # 512: dout slice per core
NKB = S // 128  # 16 k-blocks
NQC = S // 512  # 4 q-chunks
BF16 = ml_dtypes.bfloat16

PACK_HEADS = True   # pack 2 heads' score matmuls into PE row groups

_cache = {}


def _build_body(tc, nc, mybir, xT, wq, wk, wv, ow, outp):
    from concourse.masks import make_upper_triangular
    import contextlib

    dt = mybir.dt
    F = mybir.ActivationFunctionType

    pools = contextlib.ExitStack()
    tc_pool = lambda **kw: pools.enter_context(tc.tile_pool(**kw))

    singles = tc_pool(name="singles", bufs=1)
    pt_pool = tc_pool(name="pt", bufs=10)
    small = tc_pool(name="small", bufs=4)
    nq_pool = tc_pool(name="nq", bufs=12)
    ost_pool = tc_pool(name="ost", bufs=5)
    psum_st = tc_pool(name="psum_st", bufs=2, space="PSUM")
    psum_ctx = tc_pool(name="psum_ctx", bufs=2, space="PSUM")
    psum_fl = tc_pool(name="psum_fl", bufs=2, space="PSUM")

    # ---- persistent SBUF tensors. Inputs live in few merged tiles so each
    # loads with 1-2 large DMAs (HWDGE issue slots are ~625ns each); xT's
    # first 512 q/k columns get their own tile so the first projection units
    # depend on a single early DMA. ----
    xt_c0 = singles.tile([128, 8, 512], dt.bfloat16, name="xtc0")
    xt_r = singles.tile([128, 8, 3 * 512], dt.bfloat16, name="xtr")
    wq_all = singles.tile([128, 8, GD], dt.bfloat16, name="wqa")
    wk_all = singles.tile([128, 8, GD], dt.bfloat16, name="wka")
    wv_all = singles.tile([128, 8, GD], dt.bfloat16, name="wva")
    ow_all = singles.tile([128, 4, D], dt.bfloat16, name="owa")
    wq_sb = [wq_all[:, t, :] for t in range(8)]
    wk_sb = [wk_all[:, t, :] for t in range(8)]
    wv_sb = [wv_all[:, t, :] for t in range(8)]
    ow_sb = [ow_all[:, t, :] for t in range(4)]

    def xt_cols(din, lo, hi):
        """xT tile row `din`, columns [lo, hi) across the c0/rest split."""
        assert lo >= 512 or hi <= 512
        if hi <= 512:
            return xt_c0[:, din, lo:hi]
        return xt_r[:, din, lo - 512:hi - 512]
    qt_sb = [singles.tile([128, S], dt.bfloat16, name=f"qt{t}")
             for t in range(4)]                              # 2 heads / dtile
    kt_sb = [singles.tile([128, S], dt.bfloat16, name=f"kt{t}")
             for t in range(4)]
    vo_sb = [singles.tile([128, GH, 65], dt.bfloat16, name=f"vo{t}")
             for t in range(NKB)]                            # [V_h | ones]
    # normalized ctx^T tiles [128 = 2 heads x 64 d, 128 q] per (pair, seq128)
    cxtq_sb = {(hp, sq): singles.tile([128, 128], dt.bfloat16,
                                      name=f"cq{hp}_{sq}")
               for hp in range(4) for sq in range(NKB)}
    tri = singles.tile([128, 128], dt.bfloat16)              # keep k<=q

    make_upper_triangular(nc, tri, val=1.0, diag=True)
    for t in range(NKB):
        nc.vector.memset(vo_sb[t][:, :, 64:65], 1.0)

    # ---- input DMAs, ordered by first consumption, in half-tensor chunks
    # so the first projection units' low-din matmuls start ~4.5us in. ----
    xT_r = xT.ap().rearrange("(t p) s -> p t s", p=128)
    wq_r = wq.ap().rearrange("(t p) n -> p t n", p=128)
    wk_r = wk.ap().rearrange("(t p) n -> p t n", p=128)
    wv_r = wv.ap().rearrange("(t p) n -> p t n", p=128)
    ow_r = ow.ap().rearrange("(t p) n -> p t n", p=128)
    # head-pair-0 weight columns + the first 512 x columns first: that is
    # the entire critical path to the first exp (~1.5MB).
    nc.sync.dma_start(out=wq_all[:, :, 0:128], in_=wq_r[:, :, 0:128])
    nc.scalar.dma_start(out=wk_all[:, :, 0:128], in_=wk_r[:, :, 0:128])
    for h in range(4):
        ts = slice(2 * h, 2 * h + 2)
        nc.sync.dma_start(out=xt_c0[:, ts, :], in_=xT_r[:, ts, 0:512])
    nc.sync.dma_start(out=wv_all, in_=wv_r)
    nc.sync.dma_start(out=xt_r[:, :, 0:512], in_=xT_r[:, :, 512:1024])
    nc.sync.dma_start(out=xt_r[:, :, 512:1536], in_=xT_r[:, :, 1024:2048])
    nc.sync.dma_start(out=wq_all[:, :, 128:512], in_=wq_r[:, :, 128:512])
    nc.sync.dma_start(out=wk_all[:, :, 128:512], in_=wk_r[:, :, 128:512])
    nc.sync.dma_start(out=ow_all, in_=ow_r)

    # ================= filler units (PE work with no ACT dependency) ======
    # Each unit is a single [128, 512]-output psum bank accumulated over its
    # full contraction + one DVE drain copy. Units are interleaved between
    # attention iterations so the PE stays busy while the ACT engine (the
    # per-iteration bottleneck: 2 exps ~ 2us vs ~1.3us of PE work) churns.
    W_TILES = {"q": wq_sb, "k": wk_sb}
    T_TILES = {"q": qt_sb, "k": kt_sb}

    def emit_unit_fn(key):
        kind = key[0]
        if kind in ("q", "k"):
            _, hp, c = key
            ps = psum_fl.tile([128, 512], dt.float32, name="flps")
            for din in range(8):
                nc.tensor.matmul(
                    ps,
                    lhsT=W_TILES[kind][din][:, hp * 128:(hp + 1) * 128],
                    rhs=xt_cols(din, c * 512, (c + 1) * 512),
                    start=(din == 0),
                    stop=(din == 7),
                )
            nc.vector.tensor_copy(
                out=T_TILES[kind][hp][:, c * 512:(c + 1) * 512], in_=ps)
        elif kind == "v":
            _, st = key
            ps = psum_fl.tile([128, 512], dt.float32, name="flps")
            for din in range(8):
                nc.tensor.matmul(
                    ps,
                    lhsT=xt_cols(din, st * 128, st * 128 + 128),
                    rhs=wv_sb[din],
                    start=(din == 0),
                    stop=(din == 7),
                )
            nc.vector.tensor_copy(
                out=vo_sb[st][:, :, 0:64],
                in_=ps.rearrange("p (h d) -> p h d", h=GH),
            )
        else:  # ("p4", sq, mode): full [128, 1024] out row in bf16
            _, sq, mode = key
            ost = ost_pool.tile([128, 1024], dt.bfloat16, name="ost")
            if mode == 1:  # tail: scores done, use the wide score psum;
                # half copies + half DMAs so the write-out starts early
                ps = psum_st.tile([128, 1024], dt.float32, name="stp")
                for oc in range(2):
                    for dvt in range(4):
                        nc.tensor.matmul(
                            ps[:, oc * 512:(oc + 1) * 512],
                            lhsT=cxtq_sb[(dvt, sq)],
                            rhs=ow_sb[dvt][:, oc * 512:(oc + 1) * 512],
                            start=(dvt == 0),
                            stop=(dvt == 3),
                        )
                    nc.vector.tensor_copy(
                        out=ost[:, oc * 512:(oc + 1) * 512],
                        in_=ps[:, oc * 512:(oc + 1) * 512])
                    nc.sync.dma_start(
                        out=outp.ap()[sq * 128:(sq + 1) * 128,
                                      oc * 512:(oc + 1) * 512],
                        in_=ost[:, oc * 512:(oc + 1) * 512])
            else:
                for oc in range(2):
                    ps = psum_fl.tile([128, 512], dt.float32, name="flps")
                    for dvt in range(4):
                        nc.tensor.matmul(
                            ps,
                            lhsT=cxtq_sb[(dvt, sq)],
                            rhs=ow_sb[dvt][:, oc * 512:(oc + 1) * 512],
                            start=(dvt == 0),
                            stop=(dvt == 3),
                        )
                    nc.vector.tensor_copy(
                        out=ost[:, oc * 512:(oc + 1) * 512], in_=ps)
                nc.sync.dma_start(
                    out=outp.ap()[sq * 128:(sq + 1) * 128, :], in_=ost)

    UNIT_PE = {"q": 1707, "k": 1707, "v": 1707, "p4": 1707}
    emitted = set()
    lazy_q = []
    # bal tracks the LOCAL PE-idle credit (ACT minus PE time of recent
    # iterations). It is clamped from below: a burst of forced units leaves
    # only a bounded PE backlog because the exp stream itself stalls behind
    # the PE and the two streams re-synchronize.
    clock = {"bal": 0.0}

    def emit_unit(key):
        canon = key[:2] if key[0] == "p4" else key
        if canon in emitted:
            return
        emitted.add(canon)
        emit_unit_fn(key)
        clock["bal"] = max(clock["bal"] - UNIT_PE[key[0]], 0.0)

    def pop_lazy():
        while lazy_q:
            key = lazy_q[0]
            if (key[:2] if key[0] == "p4" else key) in emitted:
                lazy_q.pop(0)
                continue
            if clock["bal"] < UNIT_PE[key[0]]:
                break
            emit_unit(lazy_q.pop(0))

    # ===================== attention block =================================
    def attn_block(qc, hp, fast_drain=False, on_sq_ready=None):
        """512-wide q chunk qc for heads h0=2*hp (PE rows 0:64) and
        h1=2*hp+1 (rows 64:128).

        Two k-blocks share one [128,1024] ST psum tile so each exp covers
        ~1024 cols; the causal mask is a -1e9 accumulating matmul on the
        diagonal blocks; PV (PT as lhsT, [V|1] as rhs, out [128 q, 65])
        trails two iterations so the PE never waits on the exp. Yields
        between k-block-pair iterations so the driver can interleave filler
        units while the ACT engine is the per-iteration bottleneck."""
        nkb = 4 * qc + 4
        q0 = 512 * qc
        ctxs = [psum_ctx.tile([128, 512], dt.float32, name="ctx")
                for _ in range(2)]
        started = [False, False]
        pend2 = []
        done_j = 0  # qsb's fully accumulated (and, if fast_drain, normed)

        def norm_and_transpose(ja, jb):
            """Normalize qsb's [ja, jb) of both halves and transpose out."""
            nqs = {j: nq_pool.tile([128, 128], dt.bfloat16, name="nq")
                   for j in range(ja, jb)}
            for half in range(2):
                ctx_t = ctxs[half]
                dens = ctx_t[:, 0:260].rearrange(
                    "p (j c) -> p c j", c=65)[:, 64, ja:jb]
                recip = small.tile([128, jb - ja], dt.float32, name="recip")
                nc.vector.reciprocal(out=recip, in_=dens)
                for j in range(ja, jb):
                    nc.vector.tensor_scalar_mul(
                        nqs[j][:, half * 64:half * 64 + 64],
                        ctx_t[:, j * 65:j * 65 + 64],
                        recip[:, j - ja:j - ja + 1],
                    )
            for j in range(ja, jb):
                nc.sync.dma_start_transpose(
                    out=cxtq_sb[(hp, 4 * qc + j)], in_=nqs[j])
                if on_sq_ready is not None:
                    on_sq_ready(4 * qc + j)

        def emit_pv(kbs, offs, ns, pts):
            nonlocal done_j
            for half in range(2):
                for (kb, off, n) in zip(kbs, offs, ns):
                    j0 = (512 - n) // 128
                    for j in range(j0, 4):
                        c0 = off + (j - j0) * 128
                        nc.tensor.matmul(
                            ctxs[half][:, j * 65:j * 65 + 65],
                            lhsT=pts[half][:, c0:c0 + 128],
                            rhs=vo_sb[kb][:, 2 * hp + half, :],
                            start=not started[half],
                            stop=(kb == nkb - 1 and j == 3),
                        )
                        started[half] = True
            if fast_drain:
                new_done = max(kb - 4 * qc + 1 for kb in kbs) \
                    if kbs[-1] >= 4 * qc else 0
                if new_done > done_j:
                    norm_and_transpose(done_j, new_done)
                    done_j = new_done

        for kb0 in range(0, nkb, 2):
            kbs = [kb for kb in (kb0, kb0 + 1) if kb < nkb]
            ns = [512 - max(0, kb * 128 - q0) for kb in kbs]
            offs = [0] + [ns[0]] * (len(kbs) - 1)
            pts = []
            for half in range(2):
                p0 = half * 64
                stp = psum_st.tile([128, 1024], dt.float32, name="stp")
                for kb, off, n in zip(kbs, offs, ns):
                    nc.tensor.matmul(
                        stp[:, off:off + n],
                        lhsT=kt_sb[hp][p0:p0 + 64, kb * 128:(kb + 1) * 128],
                        rhs=qt_sb[hp][p0:p0 + 64, q0 + 512 - n:q0 + 512],
                        start=True,
                        stop=True,
                        tile_position=(p0, 0) if PACK_HEADS else None,
                    )
                ntot = offs[-1] + ns[-1]
                pt = pt_pool.tile([128, 1024], dt.bfloat16, name="pt")
                nc.scalar.activation(
                    out=pt[:, :ntot], in_=stp[:, :ntot], func=F.Exp,
                    scale=0.125)
                for kb, off in zip(kbs, offs):
                    if kb >= 4 * qc:  # diagonal: mask first 128 cols
                        nc.vector.tensor_mul(
                            pt[:, off:off + 128], pt[:, off:off + 128], tri)
                pts.append(pt)
            pend2.append((kbs, offs, ns, pts))
            if len(pend2) > (1 if fast_drain else 4):
                emit_pv(*pend2.pop(0))
            ntot = offs[-1] + ns[-1]
            nqsb = sum(4 - (512 - n) // 128 for n in ns)
            clock["bal"] += 2 * (ntot * 0.8333 + 190) \
                - (2 * ntot + 65 * 2 * nqsb) * 0.4167
            yield
        for p in pend2:
            emit_pv(*p)
        if done_j < 4:
            norm_and_transpose(done_j, 4)

    # ===================== driver ==========================================
    # Anti-diagonal wavefront over (head-pair, q-chunk): each head-pair's
    # q-chunks still run in order (kt accumulates per chunk), but head-pairs
    # are staggered so sq groups finish progressively and the out-proj fills
    # the late ACT-bound windows instead of piling into a tail. QK/V units
    # for later blocks fill the PE between attention iterations, paced
    # against a simple ACT-vs-PE clock; deadline units are forced.
    ORDER = [(0, 0), (0, 1), (1, 0), (0, 2), (1, 1), (2, 0), (0, 3), (1, 2),
             (2, 1), (3, 0), (1, 3), (2, 2), (3, 1), (2, 3), (3, 2), (3, 3)]
    vseen = set()
    for hp, qc in ORDER:
        lazy_q.append(("q", hp, qc))
        lazy_q.append(("k", hp, qc))
        if qc not in vseen:
            vseen.add(qc)
            lazy_q.extend(("v", st) for st in range(4 * qc, 4 * qc + 4))

    def last_sq_ready(sq):
        # the final block: out-proj of each finished sq inline (the filler
        # psum pool is free by then).
        emit_unit(("p4", sq, 0))

    def emit_first_qk():
        """First q/k units with din-halves interleaved to track the
        half-tensor input DMAs, so the PE streams as data arrives."""
        psq = psum_fl.tile([128, 512], dt.float32, name="flps")
        psk = psum_fl.tile([128, 512], dt.float32, name="flps")
        for h in range(4):
            for kind, ps in (("q", psq), ("k", psk)):
                for din in range(2 * h, 2 * h + 2):
                    nc.tensor.matmul(
                        ps,
                        lhsT=W_TILES[kind][din][:, 0:128],
                        rhs=xt_cols(din, 0, 512),
                        start=(din == 0),
                        stop=(din == 7),
                    )
        nc.vector.tensor_copy(out=qt_sb[0][:, 0:512], in_=psq)
        nc.vector.tensor_copy(out=kt_sb[0][:, 0:512], in_=psk)
        emitted.add(("q", 0, 0))
        emitted.add(("k", 0, 0))
        clock["bal"] = max(clock["bal"] - 2 * UNIT_PE["q"], -3000.0)

    emit_first_qk()
    for hp, qc in ORDER:
        last = (hp, qc) == ORDER[-1]
        if hp == 3 and qc >= 2:
            # late hp3 blocks: little follows — drain the lazy queue into
            # their ACT-bound iterations instead of a post-exp tail
            clock["bal"] += 2500.0
        emit_unit(("q", hp, qc))
        emit_unit(("k", hp, qc))
        it = attn_block(qc, hp, fast_drain=last,
                        on_sq_ready=last_sq_ready if last else None)
        for i, _ in enumerate(it):
            if i == 0:
                # vo tiles are only read by the PV matmuls, which trail the
                # scores by 2 iterations — forcing V here keeps the first
                # scores off the wv-DMA critical path.
                for st in range(4 * qc, 4 * qc + 4):
                    emit_unit(("v", st))
            pop_lazy()
        if hp == 3:  # sq group qc now has all head-pairs' ctx
            for sq in range(4 * qc, 4 * qc + 4):
                lazy_q.append(("p4", sq, 0))
    import sys as _sys
    print("tail-leftover p4:", [sq for sq in range(NKB)
                                if ("p4", sq) not in emitted],
          file=_sys.stderr)
    # tail: alternate the wide score psum and the filler pool so three
    # psum pairs pipeline the drain copies
    for i, sq in enumerate(range(NKB)):
        emit_unit(("p4", sq, 1 if i % 2 == 0 else 0))

    return pools


def _build_nc():
    import concourse.tile as tile
    from concourse import bacc, mybir

    dt = mybir.dt
    nc = bacc.Bacc("TRN2", target_bir_lowering=False, debug=False,
                   num_devices=NCORES)
    xT = nc.dram_tensor("xt", [D, S], dt.bfloat16, kind="ExternalInput")
    wq = nc.dram_tensor("wq", [D, GD], dt.bfloat16, kind="ExternalInput")
    wk = nc.dram_tensor("wk", [D, GD], dt.bfloat16, kind="ExternalInput")
    wv = nc.dram_tensor("wv", [D, GD], dt.bfloat16, kind="ExternalInput")
    ow = nc.dram_tensor("ow", [GD, D], dt.bfloat16, kind="ExternalInput")
    outp = nc.dram_tensor("outp", [S, D], dt.bfloat16, kind="ExternalOutput")

    with tile.TileContext(nc) as tc:
        pools = _build_body(tc, nc, mybir, xT, wq, wk, wv, ow, outp)
        pools.close()
    nc.compile()
    return nc


LAST_RESULTS = None


def kernel(batch, w_query, w_key, w_value, out_w, out_b):
    global LAST_RESULTS
    import os
    from concourse import bass_utils

    try:  # BASS_TRACE needs the axon NTFF hook; without it the run crashes
        from antenv.axon_hooks import get_axon_ntff_profile_hook  # noqa: F401
    except ImportError:
        os.environ.setdefault("BASS_NEVER_TRACE", "1")

    batch = np.asarray(batch, dtype=np.float32)
    w_query = np.asarray(w_query, dtype=np.float32)
    w_key = np.asarray(w_key, dtype=np.float32)
    w_value = np.asarray(w_value, dtype=np.float32)
    out_w = np.asarray(out_w, dtype=np.float32)
    out_b = np.asarray(out_b, dtype=np.float32)

    if "nc" not in _cache:
        _cache["nc"] = _build_nc()
    nc = _cache["nc"]

    xts = [np.ascontiguousarray(batch[b].T).astype(BF16) for b in range(B)]
    slc = [slice(g * GD, (g + 1) * GD) for g in range(2)]
    wqs = [np.ascontiguousarray(w_query[:, s]).astype(BF16) for s in slc]
    wks = [np.ascontiguousarray(w_key[:, s]).astype(BF16) for s in slc]
    wvs = [np.ascontiguousarray(w_value[:, s]).astype(BF16) for s in slc]
    ows = [np.ascontiguousarray(out_w[s, :]).astype(BF16) for s in slc]
    in_maps = []
    for c in range(NCORES):
        b, g = divmod(c, 2)
        in_maps.append({
            "xt": xts[b], "wq": wqs[g], "wk": wks[g],
            "wv": wvs[g], "ow": ows[g],
        })

    res = bass_utils.run_bass_kernel_spmd(
        nc, in_maps, core_ids=list(range(NCORES)),
    )
    LAST_RESULTS = res

    out = np.empty((B, S, D), np.float32)
    for b in range(B):
        out[b] = (res.results[2 * b]["outp"].astype(np.float32)
                  + res.results[2 * b + 1]["outp"].astype(np.float32)
                  + out_b[None, :])
    return out


# revision 72
# speedup vs baseline: 1.0031x; 1.0031x over previous
"""Multi-head causal attention (B=4, S=2048, D=1024, 16 heads) on 8 TRN2 cores.

Sharding: core c -> (batch b = c//2, head-group g = c%2). Each core computes
8 heads of one batch element end-to-end (QKV proj, causal softmax attention,
out-proj rows for its head slice). Host sums the two head-group partials per
batch and adds the output bias.

Per-core pipeline (all matmuls contraction-on-partitions, bf16 in / f32 psum):
  QT/KT[dtile] = (x @ w)^T   [128p = 2 heads x 64, S]
  Vones[kb]    = [V | 1]     [128p = k, h, 65]
  attention per (512-wide q-chunk, head-pair); the pair's score matmuls are
  packed into PE row groups via tile_position; two k-blocks share one
  [128,1024] score psum so each exp covers ~1024 cols; PV matmuls trail two
  iterations behind so PE never waits on the exp:
    ST[k,q] = KT.T @ QT; PT = exp(ST/8) bf16; tri-mask on diagonal 128 cols
    ctx[q 128, h, 65] += PT.T @ [V_h | 1]  (PT slice is the lhsT: output has
      q on partitions -> full 128-row PE output, ~half the PV stream cols of
      the [65, q] orientation; col 64 = softmax denominator)
    nq[q, d] = ctx * (1/denominator)  per-partition scale on DVE
    cxtq[d, q] = DMA-xbar-transpose(nq)  per (head-pair, 128-q tile)
  out[seq128, 512] = cxtq.T @ ow, streamed to DRAM per q-chunk; the final
  q-chunk's out-proj borrows the freed score psum banks.
"""

import numpy as np
import ml_dtypes

B, S, D = 4, 2048, 1024
H_TOT = 16
HD = 64
NCORES = 8
GH = 8          # heads per core
GD = GH * HD    # 512: dout slice per core
NKB = S // 128  # 16 k-blocks
NQC = S // 512  # 4 q-chunks
BF16 = ml_dtypes.bfloat16

PACK_HEADS = True   # pack 2 heads' score matmuls into PE row groups

_cache = {}


def _build_body(tc, nc, mybir, xT, wq, wk, wv, ow, outp):
    from concourse.masks import make_upper_triangular
    import contextlib

    dt = mybir.dt
    F = mybir.ActivationFunctionType

    pools = contextlib.ExitStack()
    tc_pool = lambda **kw: pools.enter_context(tc.tile_pool(**kw))

    singles = tc_pool(name="singles", bufs=1)
    pt_pool = tc_pool(name="pt", bufs=10)
    small = tc_pool(name="small", bufs=4)
    nq_pool = tc_pool(name="nq", bufs=12)
    ost_pool = tc_pool(name="ost", bufs=5)
    psum_st = tc_pool(name="psum_st", bufs=2, space="PSUM")
    psum_ctx = tc_pool(name="psum_ctx", bufs=2, space="PSUM")
    psum_fl = tc_pool(name="psum_fl", bufs=2, space="PSUM")

    # ---- persistent SBUF tensors. Inputs live in few merged tiles so each
    # loads with 1-2 large DMAs (HWDGE issue slots are ~625ns each); xT's
    # first 512 q/k columns get their own tile so the first projection units
    # depend on a single early DMA. ----
    xt_c0 = singles.tile([128, 8, 512], dt.bfloat16, name="xtc0")
    xt_r = singles.tile([128, 8, 3 * 512], dt.bfloat16, name="xtr")
    wq_all = singles.tile([128, 8, GD], dt.bfloat16, name="wqa")
    wk_all = singles.tile([128, 8, GD], dt.bfloat16, name="wka")
    wv_all = singles.tile([128, 8, GD], dt.bfloat16, name="wva")
    ow_all = singles.tile([128, 4, D], dt.bfloat16, name="owa")
    wq_sb = [wq_all[:, t, :] for t in range(8)]
    wk_sb = [wk_all[:, t, :] for t in range(8)]
    wv_sb = [wv_all[:, t, :] for t in range(8)]
    ow_sb = [ow_all[:, t, :] for t in range(4)]

    def xt_cols(din, lo, hi):
        """xT tile row `din`, columns [lo, hi) across the c0/rest split."""
        assert lo >= 512 or hi <= 512
        if hi <= 512:
            return xt_c0[:, din, lo:hi]
        return xt_r[:, din, lo - 512:hi - 512]
    qt_sb = [singles.tile([128, S], dt.bfloat16, name=f"qt{t}")
             for t in range(4)]                              # 2 heads / dtile
    kt_sb = [singles.tile([128, S], dt.bfloat16, name=f"kt{t}")
             for t in range(4)]
    vo_sb = [singles.tile([128, GH, 65], dt.bfloat16, name=f"vo{t}")
             for t in range(NKB)]                            # [V_h | ones]
    # normalized ctx^T tiles [128 = 2 heads x 64 d, 128 q] per (pair, seq128)
    cxtq_sb = {(hp, sq): singles.tile([128, 128], dt.bfloat16,
                                      name=f"cq{hp}_{sq}")
               for hp in range(4) for sq in range(NKB)}
    tri = singles.tile([128, 128], dt.bfloat16)              # keep k<=q

    make_upper_triangular(nc, tri, val=1.0, diag=True)
    for t in range(NKB):
        nc.vector.memset(vo_sb[t][:, :, 64:65], 1.0)

    # ---- input DMAs, ordered by first consumption, in half-tensor chunks
    # so the first projection units' low-din matmuls start ~4.5us in. ----
    xT_r = xT.ap().rearrange("(t p) s -> p t s", p=128)
    wq_r = wq.ap().rearrange("(t p) n -> p t n", p=128)
    wk_r = wk.ap().rearrange("(t p) n -> p t n", p=128)
    wv_r = wv.ap().rearrange("(t p) n -> p t n", p=128)
    ow_r = ow.ap().rearrange("(t p) n -> p t n", p=128)
    # head-pair-0 weight columns + the first 512 x columns first: that is
    # the entire critical path to the first exp (~1.5MB).
    nc.sync.dma_start(out=wq_all[:, :, 0:128], in_=wq_r[:, :, 0:128])
    nc.scalar.dma_start(out=wk_all[:, :, 0:128], in_=wk_r[:, :, 0:128])
    for h in range(4):
        ts = slice(2 * h, 2 * h + 2)
        nc.sync.dma_start(out=xt_c0[:, ts, :], in_=xT_r[:, ts, 0:512])
    nc.sync.dma_start(out=wv_all, in_=wv_r)
    nc.sync.dma_start(out=xt_r[:, :, 0:512], in_=xT_r[:, :, 512:1024])
    nc.sync.dma_start(out=xt_r[:, :, 512:1536], in_=xT_r[:, :, 1024:2048])
    nc.sync.dma_start(out=wq_all[:, :, 128:512], in_=wq_r[:, :, 128:512])
    nc.sync.dma_start(out=wk_all[:, :, 128:512], in_=wk_r[:, :, 128:512])
    nc.sync.dma_start(out=ow_all, in_=ow_r)

    # ================= filler units (PE work with no ACT dependency) ======
    # Each unit is a single [128, 512]-output psum bank accumulated over its
    # full contraction + one DVE drain copy. Units are interleaved between
    # attention iterations so the PE stays busy while the ACT engine (the
    # per-iteration bottleneck: 2 exps ~ 2us vs ~1.3us of PE work) churns.
    W_TILES = {"q": wq_sb, "k": wk_sb}
    T_TILES = {"q": qt_sb, "k": kt_sb}

    def emit_unit_fn(key):
        kind = key[0]
        if kind in ("q", "k"):
            _, hp, c = key
            ps = psum_fl.tile([128, 512], dt.float32, name="flps")
            for din in range(8):
                nc.tensor.matmul(
                    ps,
                    lhsT=W_TILES[kind][din][:, hp * 128:(hp + 1) * 128],
                    rhs=xt_cols(din, c * 512, (c + 1) * 512),
                    start=(din == 0),
                    stop=(din == 7),
                )
            nc.vector.tensor_copy(
                out=T_TILES[kind][hp][:, c * 512:(c + 1) * 512], in_=ps)
        elif kind == "v":
            _, st = key
            ps = psum_fl.tile([128, 512], dt.float32, name="flps")
            for din in range(8):
                nc.tensor.matmul(
                    ps,
                    lhsT=xt_cols(din, st * 128, st * 128 + 128),
                    rhs=wv_sb[din],
                    start=(din == 0),
                    stop=(din == 7),
                )
            nc.vector.tensor_copy(
                out=vo_sb[st][:, :, 0:64],
                in_=ps.rearrange("p (h d) -> p h d", h=GH),
            )
        else:  # ("p4", sq, mode): full [128, 1024] out row in bf16
            _, sq, mode = key
            ost = ost_pool.tile([128, 1024], dt.bfloat16, name="ost")
            if mode == 1:  # tail: scores done, use the wide score psum;
                # half copies + half DMAs so the write-out starts early
                ps = psum_st.tile([128, 1024], dt.float32, name="stp")
                for oc in range(2):
                    for dvt in range(4):
                        nc.tensor.matmul(
                            ps[:, oc * 512:(oc + 1) * 512],
                            lhsT=cxtq_sb[(dvt, sq)],
                            rhs=ow_sb[dvt][:, oc * 512:(oc + 1) * 512],
                            start=(dvt == 0),
                            stop=(dvt == 3),
                        )
                    nc.vector.tensor_copy(
                        out=ost[:, oc * 512:(oc + 1) * 512],
                        in_=ps[:, oc * 512:(oc + 1) * 512])
                    nc.sync.dma_start(
                        out=outp.ap()[sq * 128:(sq + 1) * 128,
                                      oc * 512:(oc + 1) * 512],
                        in_=ost[:, oc * 512:(oc + 1) * 512])
            else:
                for oc in range(2):
                    ps = psum_fl.tile([128, 512], dt.float32, name="flps")
                    for dvt in range(4):
                        nc.tensor.matmul(
                            ps,
                            lhsT=cxtq_sb[(dvt, sq)],
                            rhs=ow_sb[dvt][:, oc * 512:(oc + 1) * 512],
                            start=(dvt == 0),
                            stop=(dvt == 3),
                        )
                    nc.vector.tensor_copy(
                        out=ost[:, oc * 512:(oc + 1) * 512], in_=ps)
                nc.sync.dma_start(
                    out=outp.ap()[sq * 128:(sq + 1) * 128, :], in_=ost)

    UNIT_PE = {"q": 1707, "k": 1707, "v": 1707, "p4": 1707}
    emitted = set()
    lazy_q = []
    # bal tracks the LOCAL PE-idle credit (ACT minus PE time of recent
    # iterations). It is clamped from below: a burst of forced units leaves
    # only a bounded PE backlog because the exp stream itself stalls behind
    # the PE and the two streams re-synchronize.
    clock = {"bal": 0.0}

    def emit_unit(key):
        canon = key[:2] if key[0] == "p4" else key
        if canon in emitted:
            return
        emitted.add(canon)
        emit_unit_fn(key)
        clock["bal"] = max(clock["bal"] - UNIT_PE[key[0]], 0.0)

    def pop_lazy():
        while lazy_q:
            key = lazy_q[0]
            if (key[:2] if key[0] == "p4" else key) in emitted:
                lazy_q.pop(0)
                continue
            if clock["bal"] < UNIT_PE[key[0]]:
                break
            emit_unit(lazy_q.pop(0))

    # ===================== attention block =================================
    def attn_block(qc, hp, fast_drain=False, on_sq_ready=None):
        """512-wide q chunk qc for heads h0=2*hp (PE rows 0:64) and
        h1=2*hp+1 (rows 64:128).

        Two k-blocks share one [128,1024] ST psum tile so each exp covers
        ~1024 cols; the causal mask is a -1e9 accumulating matmul on the
        diagonal blocks; PV (PT as lhsT, [V|1] as rhs, out [128 q, 65])
        trails two iterations so the PE never waits on the exp. Yields
        between k-block-pair iterations so the driver can interleave filler
        units while the ACT engine is the per-iteration bottleneck."""
        nkb = 4 * qc + 4
        q0 = 512 * qc
        ctxs = [psum_ctx.tile([128, 512], dt.float32, name="ctx")
                for _ in range(2)]
        started = [False, False]
        pend2 = []
        done_j = 0  # qsb's fully accumulated (and, if fast_drain, normed)

        def norm_and_transpose(ja, jb):
            """Normalize qsb's [ja, jb) of both halves and transpose out."""
            nqs = {j: nq_pool.tile([128, 128], dt.bfloat16, name="nq")
                   for j in range(ja, jb)}
            for half in range(2):
                ctx_t = ctxs[half]
                dens = ctx_t[:, 0:260].rearrange(
                    "p (j c) -> p c j", c=65)[:, 64, ja:jb]
                recip = small.tile([128, jb - ja], dt.float32, name="recip")
                nc.vector.reciprocal(out=recip, in_=dens)
                for j in range(ja, jb):
                    nc.vector.tensor_scalar_mul(
                        nqs[j][:, half * 64:half * 64 + 64],
                        ctx_t[:, j * 65:j * 65 + 64],
                        recip[:, j - ja:j - ja + 1],
                    )
            for j in range(ja, jb):
                nc.sync.dma_start_transpose(
                    out=cxtq_sb[(hp, 4 * qc + j)], in_=nqs[j])
                if on_sq_ready is not None:
                    on_sq_ready(4 * qc + j)

        def emit_pv(kbs, offs, ns, pts):
            nonlocal done_j
            for half in range(2):
                for (kb, off, n) in zip(kbs, offs, ns):
                    j0 = (512 - n) // 128
                    for j in range(j0, 4):
                        c0 = off + (j - j0) * 128
                        nc.tensor.matmul(
                            ctxs[half][:, j * 65:j * 65 + 65],
                            lhsT=pts[half][:, c0:c0 + 128],
                            rhs=vo_sb[kb][:, 2 * hp + half, :],
                            start=not started[half],
                            stop=(kb == nkb - 1 and j == 3),
                        )
                        started[half] = True
            if fast_drain:
                new_done = max(kb - 4 * qc + 1 for kb in kbs) \
                    if kbs[-1] >= 4 * qc else 0
                if new_done > done_j:
                    norm_and_transpose(done_j, new_done)
                    done_j = new_done

        for kb0 in range(0, nkb, 2):
            kbs = [kb for kb in (kb0, kb0 + 1) if kb < nkb]
            ns = [512 - max(0, kb * 128 - q0) for kb in kbs]
            offs = [0] + [ns[0]] * (len(kbs) - 1)
            pts = []
            for half in range(2):
                p0 = half * 64
                stp = psum_st.tile([128, 1024], dt.float32, name="stp")
                for kb, off, n in zip(kbs, offs, ns):
                    nc.tensor.matmul(
                        stp[:, off:off + n],
                        lhsT=kt_sb[hp][p0:p0 + 64, kb * 128:(kb + 1) * 128],
                        rhs=qt_sb[hp][p0:p0 + 64, q0 + 512 - n:q0 + 512],
                        start=True,
                        stop=True,
                        tile_position=(p0, 0) if PACK_HEADS else None,
                    )
                ntot = offs[-1] + ns[-1]
                pt = pt_pool.tile([128, 1024], dt.bfloat16, name="pt")
                nc.scalar.activation(
                    out=pt[:, :ntot], in_=stp[:, :ntot], func=F.Exp,
                    scale=0.125)
                for kb, off in zip(kbs, offs):
                    if kb >= 4 * qc:  # diagonal: mask first 128 cols
                        nc.vector.tensor_mul(
                            pt[:, off:off + 128], pt[:, off:off + 128], tri)
                pts.append(pt)
            pend2.append((kbs, offs, ns, pts))
            if len(pend2) > (1 if fast_drain else 4):
                emit_pv(*pend2.pop(0))
            ntot = offs[-1] + ns[-1]
            nqsb = sum(4 - (512 - n) // 128 for n in ns)
            clock["bal"] += 2 * (ntot * 0.8333 + 190) \
                - (2 * ntot + 65 * 2 * nqsb) * 0.4167
            yield
        for p in pend2:
            emit_pv(*p)
        if done_j < 4:
            norm_and_transpose(done_j, 4)

    # ===================== driver ==========================================
    # Anti-diagonal wavefront over (head-pair, q-chunk): each head-pair's
    # q-chunks still run in order (kt accumulates per chunk), but head-pairs
    # are staggered so sq groups finish progressively and the out-proj fills
    # the late ACT-bound windows instead of piling into a tail. QK/V units
    # for later blocks fill the PE between attention iterations, paced
    # against a simple ACT-vs-PE clock; deadline units are forced.
    ORDER = [(0, 0), (0, 1), (1, 0), (0, 2), (1, 1), (2, 0), (0, 3), (1, 2),
             (2, 1), (3, 0), (1, 3), (2, 2), (3, 1), (2, 3), (3, 2), (3, 3)]
    vseen = set()
    for hp, qc in ORDER:
        lazy_q.append(("q", hp, qc))
        lazy_q.append(("k", hp, qc))
        if qc not in vseen:
            vseen.add(qc)
            lazy_q.extend(("v", st) for st in range(4 * qc, 4 * qc + 4))

    def last_sq_ready(sq):
        # the final block: out-proj of each finished sq inline (the filler
        # psum pool is free by then).
        emit_unit(("p4", sq, 0))

    def emit_first_qk():
        """First q/k units with din-halves interleaved to track the
        half-tensor input DMAs, so the PE streams as data arrives."""
        psq = psum_fl.tile([128, 512], dt.float32, name="flps")
        psk = psum_fl.tile([128, 512], dt.float32, name="flps")
        for h in range(4):
            for kind, ps in (("q", psq), ("k", psk)):
                for din in range(2 * h, 2 * h + 2):
                    nc.tensor.matmul(
                        ps,
                        lhsT=W_TILES[kind][din][:, 0:128],
                        rhs=xt_cols(din, 0, 512),
                        start=(din == 0),
                        stop=(din == 7),
                    )
        nc.vector.tensor_copy(out=qt_sb[0][:, 0:512], in_=psq)
        nc.vector.tensor_copy(out=kt_sb[0][:, 0:512], in_=psk)
        emitted.add(("q", 0, 0))
        emitted.add(("k", 0, 0))
        clock["bal"] = max(clock["bal"] - 2 * UNIT_PE["q"], -3000.0)

    emit_first_qk()
    for hp, qc in ORDER:
        last = (hp, qc) == ORDER[-1]
        if last:
            # final block: nothing follows — drain the lazy queue into its
            # ACT-bound iterations instead of a post-exp tail
            clock["bal"] += 3500.0
        emit_unit(("q", hp, qc))
        emit_unit(("k", hp, qc))
        it = attn_block(qc, hp, fast_drain=last,
                        on_sq_ready=last_sq_ready if last else None)
        for i, _ in enumerate(it):
            if i == 0:
                # vo tiles are only read by the PV matmuls, which trail the
                # scores by 2 iterations — forcing V here keeps the first
                # scores off the wv-DMA critical path.
                for st in range(4 * qc, 4 * qc + 4):
                    emit_unit(("v", st))
            pop_lazy()
        if hp == 3:  # sq group qc now has all head-pairs' ctx
            for sq in range(4 * qc, 4 * qc + 4):
                lazy_q.append(("p4", sq, 0))
    import sys as _sys
    print("tail-leftover p4:", [sq for sq in range(NKB)
                                if ("p4", sq) not in emitted],
          file=_sys.stderr)
    # tail: alternate the wide score psum and the filler pool so three
    # psum pairs pipeline the drain copies
    for i, sq in enumerate(range(NKB)):
        emit_unit(("p4", sq, 1 if i % 2 == 0 else 0))

    return pools


def _build_nc():
    import concourse.tile as tile
    from concourse import bacc, mybir

    dt = mybir.dt
    nc = bacc.Bacc("TRN2", target_bir_lowering=False, debug=False,
                   num_devices=NCORES)
    xT = nc.dram_tensor("xt", [D, S], dt.bfloat16, kind="ExternalInput")
    wq = nc.dram_tensor("wq", [D, GD], dt.bfloat16, kind="ExternalInput")
    wk = nc.dram_tensor("wk", [D, GD], dt.bfloat16, kind="ExternalInput")
    wv = nc.dram_tensor("wv", [D, GD], dt.bfloat16, kind="ExternalInput")
    ow = nc.dram_tensor("ow", [GD, D], dt.bfloat16, kind="ExternalInput")
    outp = nc.dram_tensor("outp", [S, D], dt.bfloat16, kind="ExternalOutput")

    with tile.TileContext(nc) as tc:
        pools = _build_body(tc, nc, mybir, xT, wq, wk, wv, ow, outp)
        pools.close()
    nc.compile()
    return nc


LAST_RESULTS = None


def kernel(batch, w_query, w_key, w_value, out_w, out_b):
    global LAST_RESULTS
    import os
    from concourse import bass_utils

    try:  # BASS_TRACE needs the axon NTFF hook; without it the run crashes
        from antenv.axon_hooks import get_axon_ntff_profile_hook  # noqa: F401
    except ImportError:
        os.environ.setdefault("BASS_NEVER_TRACE", "1")

    batch = np.asarray(batch, dtype=np.float32)
    w_query = np.asarray(w_query, dtype=np.float32)
    w_key = np.asarray(w_key, dtype=np.float32)
    w_value = np.asarray(w_value, dtype=np.float32)
    out_w = np.asarray(out_w, dtype=np.float32)
    out_b = np.asarray(out_b, dtype=np.float32)

    if "nc" not in _cache:
        _cache["nc"] = _build_nc()
    nc = _cache["nc"]

    xts = [np.ascontiguousarray(batch[b].T).astype(BF16) for b in range(B)]
    slc = [slice(g * GD, (g + 1) * GD) for g in range(2)]
    wqs = [np.ascontiguousarray(w_query[:, s]).astype(BF16) for s in slc]
    wks = [np.ascontiguousarray(w_key[:, s]).astype(BF16) for s in slc]
    wvs = [np.ascontiguousarray(w_value[:, s]).astype(BF16) for s in slc]
    ows = [np.ascontiguousarray(out_w[s, :]).astype(BF16) for s in slc]
    in_maps = []
    for c in range(NCORES):
        b, g = divmod(c, 2)
        in_maps.append({
            "xt": xts[b], "wq": wqs[g], "wk": wks[g],
            "wv": wvs[g], "ow": ows[g],
        })

    res = bass_utils.run_bass_kernel_spmd(
        nc, in_maps, core_ids=list(range(NCORES)),
    )
    LAST_RESULTS = res

    out = np.empty((B, S, D), np.float32)
    for b in range(B):
        out[b] = (res.results[2 * b]["outp"].astype(np.float32)
                  + res.results[2 * b + 1]["outp"].astype(np.float32)
                  + out_b[None, :])
    return out


# revision 73
# speedup vs baseline: 1.0039x; 1.0008x over previous
"""Multi-head causal attention (B=4, S=2048, D=1024, 16 heads) on 8 TRN2 cores.

Sharding: core c -> (batch b = c//2, head-group g = c%2). Each core computes
8 heads of one batch element end-to-end (QKV proj, causal softmax attention,
out-proj rows for its head slice). Host sums the two head-group partials per
batch and adds the output bias.

Per-core pipeline (all matmuls contraction-on-partitions, bf16 in / f32 psum):
  QT/KT[dtile] = (x @ w)^T   [128p = 2 heads x 64, S]
  Vones[kb]    = [V | 1]     [128p = k, h, 65]
  attention per (512-wide q-chunk, head-pair); the pair's score matmuls are
  packed into PE row groups via tile_position; two k-blocks share one
  [128,1024] score psum so each exp covers ~1024 cols; PV matmuls trail two
  iterations behind so PE never waits on the exp:
    ST[k,q] = KT.T @ QT; PT = exp(ST/8) bf16; tri-mask on diagonal 128 cols
    ctx[q 128, h, 65] += PT.T @ [V_h | 1]  (PT slice is the lhsT: output has
      q on partitions -> full 128-row PE output, ~half the PV stream cols of
      the [65, q] orientation; col 64 = softmax denominator)
    nq[q, d] = ctx * (1/denominator)  per-partition scale on DVE
    cxtq[d, q] = DMA-xbar-transpose(nq)  per (head-pair, 128-q tile)
  out[seq128, 512] = cxtq.T @ ow, streamed to DRAM per q-chunk; the final
  q-chunk's out-proj borrows the freed score psum banks.
"""

import numpy as np
import ml_dtypes

B, S, D = 4, 2048, 1024
H_TOT = 16
HD = 64
NCORES = 8
GH = 8          # heads per core
GD = GH * HD    # 512: dout slice per core
NKB = S // 128  # 16 k-blocks
NQC = S // 512  # 4 q-chunks
BF16 = ml_dtypes.bfloat16

PACK_HEADS = True   # pack 2 heads' score matmuls into PE row groups

_cache = {}


def _build_body(tc, nc, mybir, xT, wq, wk, wv, ow, outp):
    from concourse.masks import make_upper_triangular
    import contextlib

    dt = mybir.dt
    F = mybir.ActivationFunctionType

    pools = contextlib.ExitStack()
    tc_pool = lambda **kw: pools.enter_context(tc.tile_pool(**kw))

    singles = tc_pool(name="singles", bufs=1)
    pt_pool = tc_pool(name="pt", bufs=10)
    small = tc_pool(name="small", bufs=4)
    nq_pool = tc_pool(name="nq", bufs=12)
    ost_pool = tc_pool(name="ost", bufs=5)
    psum_st = tc_pool(name="psum_st", bufs=2, space="PSUM")
    psum_ctx = tc_pool(name="psum_ctx", bufs=2, space="PSUM")
    psum_fl = tc_pool(name="psum_fl", bufs=2, space="PSUM")

    # ---- persistent SBUF tensors. Inputs live in few merged tiles so each
    # loads with 1-2 large DMAs (HWDGE issue slots are ~625ns each); xT's
    # first 512 q/k columns get their own tile so the first projection units
    # depend on a single early DMA. ----
    xt_c0 = singles.tile([128, 8, 512], dt.bfloat16, name="xtc0")
    xt_r = singles.tile([128, 8, 3 * 512], dt.bfloat16, name="xtr")
    wq_all = singles.tile([128, 8, GD], dt.bfloat16, name="wqa")
    wk_all = singles.tile([128, 8, GD], dt.bfloat16, name="wka")
    wv_all = singles.tile([128, 8, GD], dt.bfloat16, name="wva")
    ow_all = singles.tile([128, 4, D], dt.bfloat16, name="owa")
    wq_sb = [wq_all[:, t, :] for t in range(8)]
    wk_sb = [wk_all[:, t, :] for t in range(8)]
    wv_sb = [wv_all[:, t, :] for t in range(8)]
    ow_sb = [ow_all[:, t, :] for t in range(4)]

    def xt_cols(din, lo, hi):
        """xT tile row `din`, columns [lo, hi) across the c0/rest split."""
        assert lo >= 512 or hi <= 512
        if hi <= 512:
            return xt_c0[:, din, lo:hi]
        return xt_r[:, din, lo - 512:hi - 512]
    qt_sb = [singles.tile([128, S], dt.bfloat16, name=f"qt{t}")
             for t in range(4)]                              # 2 heads / dtile
    kt_sb = [singles.tile([128, S], dt.bfloat16, name=f"kt{t}")
             for t in range(4)]
    vo_sb = [singles.tile([128, GH, 65], dt.bfloat16, name=f"vo{t}")
             for t in range(NKB)]                            # [V_h | ones]
    # normalized ctx^T tiles [128 = 2 heads x 64 d, 128 q] per (pair, seq128)
    cxtq_sb = {(hp, sq): singles.tile([128, 128], dt.bfloat16,
                                      name=f"cq{hp}_{sq}")
               for hp in range(4) for sq in range(NKB)}
    tri = singles.tile([128, 128], dt.bfloat16)              # keep k<=q

    make_upper_triangular(nc, tri, val=1.0, diag=True)
    for t in range(NKB):
        nc.vector.memset(vo_sb[t][:, :, 64:65], 1.0)

    # ---- input DMAs, ordered by first consumption, in half-tensor chunks
    # so the first projection units' low-din matmuls start ~4.5us in. ----
    xT_r = xT.ap().rearrange("(t p) s -> p t s", p=128)
    wq_r = wq.ap().rearrange("(t p) n -> p t n", p=128)
    wk_r = wk.ap().rearrange("(t p) n -> p t n", p=128)
    wv_r = wv.ap().rearrange("(t p) n -> p t n", p=128)
    ow_r = ow.ap().rearrange("(t p) n -> p t n", p=128)
    # head-pair-0 weight columns + the first 512 x columns first: that is
    # the entire critical path to the first exp (~1.5MB).
    nc.sync.dma_start(out=wq_all[:, :, 0:128], in_=wq_r[:, :, 0:128])
    nc.scalar.dma_start(out=wk_all[:, :, 0:128], in_=wk_r[:, :, 0:128])
    for h in range(4):
        ts = slice(2 * h, 2 * h + 2)
        nc.sync.dma_start(out=xt_c0[:, ts, :], in_=xT_r[:, ts, 0:512])
    nc.sync.dma_start(out=wv_all, in_=wv_r)
    nc.sync.dma_start(out=xt_r[:, :, 0:512], in_=xT_r[:, :, 512:1024])
    nc.sync.dma_start(out=xt_r[:, :, 512:1536], in_=xT_r[:, :, 1024:2048])
    nc.sync.dma_start(out=wq_all[:, :, 128:512], in_=wq_r[:, :, 128:512])
    nc.sync.dma_start(out=wk_all[:, :, 128:512], in_=wk_r[:, :, 128:512])
    nc.sync.dma_start(out=ow_all, in_=ow_r)

    # ================= filler units (PE work with no ACT dependency) ======
    # Each unit is a single [128, 512]-output psum bank accumulated over its
    # full contraction + one DVE drain copy. Units are interleaved between
    # attention iterations so the PE stays busy while the ACT engine (the
    # per-iteration bottleneck: 2 exps ~ 2us vs ~1.3us of PE work) churns.
    W_TILES = {"q": wq_sb, "k": wk_sb}
    T_TILES = {"q": qt_sb, "k": kt_sb}

    def emit_unit_fn(key):
        kind = key[0]
        if kind in ("q", "k"):
            _, hp, c = key
            ps = psum_fl.tile([128, 512], dt.float32, name="flps")
            for din in range(8):
                nc.tensor.matmul(
                    ps,
                    lhsT=W_TILES[kind][din][:, hp * 128:(hp + 1) * 128],
                    rhs=xt_cols(din, c * 512, (c + 1) * 512),
                    start=(din == 0),
                    stop=(din == 7),
                )
            nc.vector.tensor_copy(
                out=T_TILES[kind][hp][:, c * 512:(c + 1) * 512], in_=ps)
        elif kind == "v":
            _, st = key
            ps = psum_fl.tile([128, 512], dt.float32, name="flps")
            for din in range(8):
                nc.tensor.matmul(
                    ps,
                    lhsT=xt_cols(din, st * 128, st * 128 + 128),
                    rhs=wv_sb[din],
                    start=(din == 0),
                    stop=(din == 7),
                )
            nc.vector.tensor_copy(
                out=vo_sb[st][:, :, 0:64],
                in_=ps.rearrange("p (h d) -> p h d", h=GH),
            )
        else:  # ("p4", sq, mode): full [128, 1024] out row in bf16
            _, sq, mode = key
            ost = ost_pool.tile([128, 1024], dt.bfloat16, name="ost")
            if mode == 1:  # tail: scores done, use the wide score psum;
                # half copies + half DMAs so the write-out starts early
                ps = psum_st.tile([128, 1024], dt.float32, name="stp")
                for oc in range(2):
                    for dvt in range(4):
                        nc.tensor.matmul(
                            ps[:, oc * 512:(oc + 1) * 512],
                            lhsT=cxtq_sb[(dvt, sq)],
                            rhs=ow_sb[dvt][:, oc * 512:(oc + 1) * 512],
                            start=(dvt == 0),
                            stop=(dvt == 3),
                        )
                    nc.vector.tensor_copy(
                        out=ost[:, oc * 512:(oc + 1) * 512],
                        in_=ps[:, oc * 512:(oc + 1) * 512])
                    nc.sync.dma_start(
                        out=outp.ap()[sq * 128:(sq + 1) * 128,
                                      oc * 512:(oc + 1) * 512],
                        in_=ost[:, oc * 512:(oc + 1) * 512])
            else:
                for oc in range(2):
                    ps = psum_fl.tile([128, 512], dt.float32, name="flps")
                    for dvt in range(4):
                        nc.tensor.matmul(
                            ps,
                            lhsT=cxtq_sb[(dvt, sq)],
                            rhs=ow_sb[dvt][:, oc * 512:(oc + 1) * 512],
                            start=(dvt == 0),
                            stop=(dvt == 3),
                        )
                    nc.vector.tensor_copy(
                        out=ost[:, oc * 512:(oc + 1) * 512], in_=ps)
                nc.sync.dma_start(
                    out=outp.ap()[sq * 128:(sq + 1) * 128, :], in_=ost)

    UNIT_PE = {"q": 1707, "k": 1707, "v": 1707, "p4": 1707}
    emitted = set()
    lazy_q = []
    # bal tracks the LOCAL PE-idle credit (ACT minus PE time of recent
    # iterations). It is clamped from below: a burst of forced units leaves
    # only a bounded PE backlog because the exp stream itself stalls behind
    # the PE and the two streams re-synchronize.
    clock = {"bal": 0.0}

    def emit_unit(key):
        canon = key[:2] if key[0] == "p4" else key
        if canon in emitted:
            return
        emitted.add(canon)
        emit_unit_fn(key)
        clock["bal"] = max(clock["bal"] - UNIT_PE[key[0]], 0.0)

    def pop_lazy():
        while lazy_q:
            key = lazy_q[0]
            if (key[:2] if key[0] == "p4" else key) in emitted:
                lazy_q.pop(0)
                continue
            if clock["bal"] < UNIT_PE[key[0]]:
                break
            emit_unit(lazy_q.pop(0))

    # ===================== attention block =================================
    def attn_block(qc, hp, fast_drain=False, on_sq_ready=None):
        """512-wide q chunk qc for heads h0=2*hp (PE rows 0:64) and
        h1=2*hp+1 (rows 64:128).

        Two k-blocks share one [128,1024] ST psum tile so each exp covers
        ~1024 cols; the causal mask is a -1e9 accumulating matmul on the
        diagonal blocks; PV (PT as lhsT, [V|1] as rhs, out [128 q, 65])
        trails two iterations so the PE never waits on the exp. Yields
        between k-block-pair iterations so the driver can interleave filler
        units while the ACT engine is the per-iteration bottleneck."""
        nkb = 4 * qc + 4
        q0 = 512 * qc
        ctxs = [psum_ctx.tile([128, 512], dt.float32, name="ctx")
                for _ in range(2)]
        started = [False, False]
        pend2 = []
        done_j = 0  # qsb's fully accumulated (and, if fast_drain, normed)

        def norm_and_transpose(ja, jb):
            """Normalize qsb's [ja, jb) of both halves and transpose out."""
            nqs = {j: nq_pool.tile([128, 128], dt.bfloat16, name="nq")
                   for j in range(ja, jb)}
            for half in range(2):
                ctx_t = ctxs[half]
                dens = ctx_t[:, 0:260].rearrange(
                    "p (j c) -> p c j", c=65)[:, 64, ja:jb]
                recip = small.tile([128, jb - ja], dt.float32, name="recip")
                nc.vector.reciprocal(out=recip, in_=dens)
                for j in range(ja, jb):
                    nc.vector.tensor_scalar_mul(
                        nqs[j][:, half * 64:half * 64 + 64],
                        ctx_t[:, j * 65:j * 65 + 64],
                        recip[:, j - ja:j - ja + 1],
                    )
            for j in range(ja, jb):
                nc.sync.dma_start_transpose(
                    out=cxtq_sb[(hp, 4 * qc + j)], in_=nqs[j])
                if on_sq_ready is not None:
                    on_sq_ready(4 * qc + j)

        def emit_pv(kbs, offs, ns, pts):
            nonlocal done_j
            for half in range(2):
                for (kb, off, n) in zip(kbs, offs, ns):
                    j0 = (512 - n) // 128
                    for j in range(j0, 4):
                        c0 = off + (j - j0) * 128
                        nc.tensor.matmul(
                            ctxs[half][:, j * 65:j * 65 + 65],
                            lhsT=pts[half][:, c0:c0 + 128],
                            rhs=vo_sb[kb][:, 2 * hp + half, :],
                            start=not started[half],
                            stop=(kb == nkb - 1 and j == 3),
                        )
                        started[half] = True
            if fast_drain:
                new_done = max(kb - 4 * qc + 1 for kb in kbs) \
                    if kbs[-1] >= 4 * qc else 0
                if new_done > done_j:
                    norm_and_transpose(done_j, new_done)
                    done_j = new_done

        for kb0 in range(0, nkb, 2):
            kbs = [kb for kb in (kb0, kb0 + 1) if kb < nkb]
            ns = [512 - max(0, kb * 128 - q0) for kb in kbs]
            offs = [0] + [ns[0]] * (len(kbs) - 1)
            pts = []
            for half in range(2):
                p0 = half * 64
                stp = psum_st.tile([128, 1024], dt.float32, name="stp")
                for kb, off, n in zip(kbs, offs, ns):
                    nc.tensor.matmul(
                        stp[:, off:off + n],
                        lhsT=kt_sb[hp][p0:p0 + 64, kb * 128:(kb + 1) * 128],
                        rhs=qt_sb[hp][p0:p0 + 64, q0 + 512 - n:q0 + 512],
                        start=True,
                        stop=True,
                        tile_position=(p0, 0) if PACK_HEADS else None,
                    )
                ntot = offs[-1] + ns[-1]
                pt = pt_pool.tile([128, 1024], dt.bfloat16, name="pt")
                nc.scalar.activation(
                    out=pt[:, :ntot], in_=stp[:, :ntot], func=F.Exp,
                    scale=0.125)
                for kb, off in zip(kbs, offs):
                    if kb >= 4 * qc:  # diagonal: mask first 128 cols
                        nc.vector.tensor_mul(
                            pt[:, off:off + 128], pt[:, off:off + 128], tri)
                pts.append(pt)
            pend2.append((kbs, offs, ns, pts))
            if len(pend2) > (1 if fast_drain else 4):
                emit_pv(*pend2.pop(0))
            ntot = offs[-1] + ns[-1]
            nqsb = sum(4 - (512 - n) // 128 for n in ns)
            clock["bal"] += 2 * (ntot * 0.8333 + 190) \
                - (2 * ntot + 65 * 2 * nqsb) * 0.4167
            yield
        for p in pend2:
            emit_pv(*p)
        if done_j < 4:
            norm_and_transpose(done_j, 4)

    # ===================== driver ==========================================
    # Anti-diagonal wavefront over (head-pair, q-chunk): each head-pair's
    # q-chunks still run in order (kt accumulates per chunk), but head-pairs
    # are staggered so sq groups finish progressively and the out-proj fills
    # the late ACT-bound windows instead of piling into a tail. QK/V units
    # for later blocks fill the PE between attention iterations, paced
    # against a simple ACT-vs-PE clock; deadline units are forced.
    ORDER = [(0, 0), (0, 1), (1, 0), (0, 2), (1, 1), (2, 0), (0, 3), (1, 2),
             (2, 1), (3, 0), (1, 3), (2, 2), (3, 1), (2, 3), (3, 2), (3, 3)]
    vseen = set()
    for hp, qc in ORDER:
        lazy_q.append(("q", hp, qc))
        lazy_q.append(("k", hp, qc))
        if qc not in vseen:
            vseen.add(qc)
            lazy_q.extend(("v", st) for st in range(4 * qc, 4 * qc + 4))

    def last_sq_ready(sq):
        # the final block: out-proj of each finished sq inline (the filler
        # psum pool is free by then).
        emit_unit(("p4", sq, 0))

    def emit_first_qk():
        """First q/k units with din-halves interleaved to track the
        half-tensor input DMAs, so the PE streams as data arrives."""
        psq = psum_fl.tile([128, 512], dt.float32, name="flps")
        psk = psum_fl.tile([128, 512], dt.float32, name="flps")
        for h in range(4):
            for kind, ps in (("q", psq), ("k", psk)):
                for din in range(2 * h, 2 * h + 2):
                    nc.tensor.matmul(
                        ps,
                        lhsT=W_TILES[kind][din][:, 0:128],
                        rhs=xt_cols(din, 0, 512),
                        start=(din == 0),
                        stop=(din == 7),
                    )
        nc.vector.tensor_copy(out=qt_sb[0][:, 0:512], in_=psq)
        nc.vector.tensor_copy(out=kt_sb[0][:, 0:512], in_=psk)
        emitted.add(("q", 0, 0))
        emitted.add(("k", 0, 0))
        clock["bal"] = max(clock["bal"] - 2 * UNIT_PE["q"], -3000.0)

    emit_first_qk()
    for hp, qc in ORDER:
        last = (hp, qc) == ORDER[-1]
        if last:
            # final block: nothing follows — drain the lazy queue into its
            # ACT-bound iterations instead of a post-exp tail
            clock["bal"] += 2500.0
        emit_unit(("q", hp, qc))
        emit_unit(("k", hp, qc))
        it = attn_block(qc, hp, fast_drain=last,
                        on_sq_ready=last_sq_ready if last else None)
        for i, _ in enumerate(it):
            if i == 0:
                # vo tiles are only read by the PV matmuls, which trail the
                # scores by 2 iterations — forcing V here keeps the first
                # scores off the wv-DMA critical path.
                for st in range(4 * qc, 4 * qc + 4):
                    emit_unit(("v", st))
            pop_lazy()
        if hp == 3:  # sq group qc now has all head-pairs' ctx
            for sq in range(4 * qc, 4 * qc + 4):
                lazy_q.append(("p4", sq, 0))
    import sys as _sys
    print("tail-leftover p4:", [sq for sq in range(NKB)
                                if ("p4", sq) not in emitted],
          file=_sys.stderr)
    # tail: alternate the wide score psum and the filler pool so three
    # psum pairs pipeline the drain copies
    for i, sq in enumerate(range(NKB)):
        emit_unit(("p4", sq, 1 if i % 2 == 0 else 0))

    return pools


def _build_nc():
    import concourse.tile as tile
    from concourse import bacc, mybir

    dt = mybir.dt
    nc = bacc.Bacc("TRN2", target_bir_lowering=False, debug=False,
                   num_devices=NCORES)
    xT = nc.dram_tensor("xt", [D, S], dt.bfloat16, kind="ExternalInput")
    wq = nc.dram_tensor("wq", [D, GD], dt.bfloat16, kind="ExternalInput")
    wk = nc.dram_tensor("wk", [D, GD], dt.bfloat16, kind="ExternalInput")
    wv = nc.dram_tensor("wv", [D, GD], dt.bfloat16, kind="ExternalInput")
    ow = nc.dram_tensor("ow", [GD, D], dt.bfloat16, kind="ExternalInput")
    outp = nc.dram_tensor("outp", [S, D], dt.bfloat16, kind="ExternalOutput")

    with tile.TileContext(nc) as tc:
        pools = _build_body(tc, nc, mybir, xT, wq, wk, wv, ow, outp)
        pools.close()
    nc.compile()
    return nc


LAST_RESULTS = None


def kernel(batch, w_query, w_key, w_value, out_w, out_b):
    global LAST_RESULTS
    import os
    from concourse import bass_utils

    try:  # BASS_TRACE needs the axon NTFF hook; without it the run crashes
        from antenv.axon_hooks import get_axon_ntff_profile_hook  # noqa: F401
    except ImportError:
        os.environ.setdefault("BASS_NEVER_TRACE", "1")

    batch = np.asarray(batch, dtype=np.float32)
    w_query = np.asarray(w_query, dtype=np.float32)
    w_key = np.asarray(w_key, dtype=np.float32)
    w_value = np.asarray(w_value, dtype=np.float32)
    out_w = np.asarray(out_w, dtype=np.float32)
    out_b = np.asarray(out_b, dtype=np.float32)

    if "nc" not in _cache:
        _cache["nc"] = _build_nc()
    nc = _cache["nc"]

    xts = [np.ascontiguousarray(batch[b].T).astype(BF16) for b in range(B)]
    slc = [slice(g * GD, (g + 1) * GD) for g in range(2)]
    wqs = [np.ascontiguousarray(w_query[:, s]).astype(BF16) for s in slc]
    wks = [np.ascontiguousarray(w_key[:, s]).astype(BF16) for s in slc]
    wvs = [np.ascontiguousarray(w_value[:, s]).astype(BF16) for s in slc]
    ows = [np.ascontiguousarray(out_w[s, :]).astype(BF16) for s in slc]
    in_maps = []
    for c in range(NCORES):
        b, g = divmod(c, 2)
        in_maps.append({
            "xt": xts[b], "wq": wqs[g], "wk": wks[g],
            "wv": wvs[g], "ow": ows[g],
        })

    res = bass_utils.run_bass_kernel_spmd(
        nc, in_maps, core_ids=list(range(NCORES)),
    )
    LAST_RESULTS = res

    out = np.empty((B, S, D), np.float32)
    for b in range(B):
        out[b] = (res.results[2 * b]["outp"].astype(np.float32)
                  + res.results[2 * b + 1]["outp"].astype(np.float32)
                  + out_b[None, :])
    return out


# revision 74
# speedup vs baseline: 1.0044x; 1.0005x over previous
"""Multi-head causal attention (B=4, S=2048, D=1024, 16 heads) on 8 TRN2 cores.

Sharding: core c -> (batch b = c//2, head-group g = c%2). Each core computes
8 heads of one batch element end-to-end (QKV proj, causal softmax attention,
out-proj rows for its head slice). Host sums the two head-group partials per
batch and adds the output bias.

Per-core pipeline (all matmuls contraction-on-partitions, bf16 in / f32 psum):
  QT/KT[dtile] = (x @ w)^T   [128p = 2 heads x 64, S]
  Vones[kb]    = [V | 1]     [128p = k, h, 65]
  attention per (512-wide q-chunk, head-pair); the pair's score matmuls are
  packed into PE row groups via tile_position; two k-blocks share one
  [128,1024] score psum so each exp covers ~1024 cols; PV matmuls trail two
  iterations behind so PE never waits on the exp:
    ST[k,q] = KT.T @ QT; PT = exp(ST/8) bf16; tri-mask on diagonal 128 cols
    ctx[q 128, h, 65] += PT.T @ [V_h | 1]  (PT slice is the lhsT: output has
      q on partitions -> full 128-row PE output, ~half the PV stream cols of
      the [65, q] orientation; col 64 = softmax denominator)
    nq[q, d] = ctx * (1/denominator)  per-partition scale on DVE
    cxtq[d, q] = DMA-xbar-transpose(nq)  per (head-pair, 128-q tile)
  out[seq128, 512] = cxtq.T @ ow, streamed to DRAM per q-chunk; the final
  q-chunk's out-proj borrows the freed score psum banks.
"""

import numpy as np
import ml_dtypes

B, S, D = 4, 2048, 1024
H_TOT = 16
HD = 64
NCORES = 8
GH = 8          # heads per core
GD = GH * HD    # 512: dout slice per core
NKB = S // 128  # 16 k-blocks
NQC = S // 512  # 4 q-chunks
BF16 = ml_dtypes.bfloat16

PACK_HEADS = True   # pack 2 heads' score matmuls into PE row groups

_cache = {}


def _build_body(tc, nc, mybir, xT, wq, wk, wv, ow, outp):
    from concourse.masks import make_upper_triangular
    import contextlib

    dt = mybir.dt
    F = mybir.ActivationFunctionType

    pools = contextlib.ExitStack()
    tc_pool = lambda **kw: pools.enter_context(tc.tile_pool(**kw))

    singles = tc_pool(name="singles", bufs=1)
    pt_pool = tc_pool(name="pt", bufs=10)
    small = tc_pool(name="small", bufs=4)
    nq_pool = tc_pool(name="nq", bufs=12)
    ost_pool = tc_pool(name="ost", bufs=5)
    psum_st = tc_pool(name="psum_st", bufs=2, space="PSUM")
    psum_ctx = tc_pool(name="psum_ctx", bufs=2, space="PSUM")
    psum_fl = tc_pool(name="psum_fl", bufs=2, space="PSUM")

    # ---- persistent SBUF tensors. Inputs live in few merged tiles so each
    # loads with 1-2 large DMAs (HWDGE issue slots are ~625ns each); xT's
    # first 512 q/k columns get their own tile so the first projection units
    # depend on a single early DMA. ----
    xt_c0 = singles.tile([128, 8, 512], dt.bfloat16, name="xtc0")
    xt_r = singles.tile([128, 8, 3 * 512], dt.bfloat16, name="xtr")
    wq_all = singles.tile([128, 8, GD], dt.bfloat16, name="wqa")
    wk_all = singles.tile([128, 8, GD], dt.bfloat16, name="wka")
    wv_all = singles.tile([128, 8, GD], dt.bfloat16, name="wva")
    ow_all = singles.tile([128, 4, D], dt.bfloat16, name="owa")
    wq_sb = [wq_all[:, t, :] for t in range(8)]
    wk_sb = [wk_all[:, t, :] for t in range(8)]
    wv_sb = [wv_all[:, t, :] for t in range(8)]
    ow_sb = [ow_all[:, t, :] for t in range(4)]

    def xt_cols(din, lo, hi):
        """xT tile row `din`, columns [lo, hi) across the c0/rest split."""
        assert lo >= 512 or hi <= 512
        if hi <= 512:
            return xt_c0[:, din, lo:hi]
        return xt_r[:, din, lo - 512:hi - 512]
    qt_sb = [singles.tile([128, S], dt.bfloat16, name=f"qt{t}")
             for t in range(4)]                              # 2 heads / dtile
    kt_sb = [singles.tile([128, S], dt.bfloat16, name=f"kt{t}")
             for t in range(4)]
    vo_sb = [singles.tile([128, GH, 65], dt.bfloat16, name=f"vo{t}")
             for t in range(NKB)]                            # [V_h | ones]
    # normalized ctx^T tiles [128 = 2 heads x 64 d, 128 q] per (pair, seq128)
    cxtq_sb = {(hp, sq): singles.tile([128, 128], dt.bfloat16,
                                      name=f"cq{hp}_{sq}")
               for hp in range(4) for sq in range(NKB)}
    tri = singles.tile([128, 128], dt.bfloat16)              # keep k<=q

    make_upper_triangular(nc, tri, val=1.0, diag=True)
    for t in range(NKB):
        nc.vector.memset(vo_sb[t][:, :, 64:65], 1.0)

    # ---- input DMAs, ordered by first consumption, in half-tensor chunks
    # so the first projection units' low-din matmuls start ~4.5us in. ----
    xT_r = xT.ap().rearrange("(t p) s -> p t s", p=128)
    wq_r = wq.ap().rearrange("(t p) n -> p t n", p=128)
    wk_r = wk.ap().rearrange("(t p) n -> p t n", p=128)
    wv_r = wv.ap().rearrange("(t p) n -> p t n", p=128)
    ow_r = ow.ap().rearrange("(t p) n -> p t n", p=128)
    # head-pair-0 weight columns + the first 512 x columns first: that is
    # the entire critical path to the first exp (~1.5MB).
    nc.sync.dma_start(out=wq_all[:, :, 0:128], in_=wq_r[:, :, 0:128])
    nc.scalar.dma_start(out=wk_all[:, :, 0:128], in_=wk_r[:, :, 0:128])
    for h in range(4):
        ts = slice(2 * h, 2 * h + 2)
        nc.sync.dma_start(out=xt_c0[:, ts, :], in_=xT_r[:, ts, 0:512])
    nc.sync.dma_start(out=wv_all, in_=wv_r)
    nc.sync.dma_start(out=xt_r[:, :, 0:512], in_=xT_r[:, :, 512:1024])
    nc.sync.dma_start(out=xt_r[:, :, 512:1536], in_=xT_r[:, :, 1024:2048])
    nc.sync.dma_start(out=wq_all[:, :, 128:512], in_=wq_r[:, :, 128:512])
    nc.sync.dma_start(out=wk_all[:, :, 128:512], in_=wk_r[:, :, 128:512])
    nc.sync.dma_start(out=ow_all, in_=ow_r)

    # ================= filler units (PE work with no ACT dependency) ======
    # Each unit is a single [128, 512]-output psum bank accumulated over its
    # full contraction + one DVE drain copy. Units are interleaved between
    # attention iterations so the PE stays busy while the ACT engine (the
    # per-iteration bottleneck: 2 exps ~ 2us vs ~1.3us of PE work) churns.
    W_TILES = {"q": wq_sb, "k": wk_sb}
    T_TILES = {"q": qt_sb, "k": kt_sb}

    def emit_unit_fn(key):
        kind = key[0]
        if kind in ("q", "k"):
            _, hp, c = key
            ps = psum_fl.tile([128, 512], dt.float32, name="flps")
            for din in range(8):
                nc.tensor.matmul(
                    ps,
                    lhsT=W_TILES[kind][din][:, hp * 128:(hp + 1) * 128],
                    rhs=xt_cols(din, c * 512, (c + 1) * 512),
                    start=(din == 0),
                    stop=(din == 7),
                )
            nc.vector.tensor_copy(
                out=T_TILES[kind][hp][:, c * 512:(c + 1) * 512], in_=ps)
        elif kind == "v":
            _, st = key
            ps = psum_fl.tile([128, 512], dt.float32, name="flps")
            for din in range(8):
                nc.tensor.matmul(
                    ps,
                    lhsT=xt_cols(din, st * 128, st * 128 + 128),
                    rhs=wv_sb[din],
                    start=(din == 0),
                    stop=(din == 7),
                )
            nc.vector.tensor_copy(
                out=vo_sb[st][:, :, 0:64],
                in_=ps.rearrange("p (h d) -> p h d", h=GH),
            )
        else:  # ("p4", sq, mode): full [128, 1024] out row in bf16
            _, sq, mode = key
            ost = ost_pool.tile([128, 1024], dt.bfloat16, name="ost")
            if mode == 1:  # tail: scores done, use the wide score psum;
                # half copies + half DMAs so the write-out starts early
                ps = psum_st.tile([128, 1024], dt.float32, name="stp")
                for oc in range(2):
                    for dvt in range(4):
                        nc.tensor.matmul(
                            ps[:, oc * 512:(oc + 1) * 512],
                            lhsT=cxtq_sb[(dvt, sq)],
                            rhs=ow_sb[dvt][:, oc * 512:(oc + 1) * 512],
                            start=(dvt == 0),
                            stop=(dvt == 3),
                        )
                    nc.vector.tensor_copy(
                        out=ost[:, oc * 512:(oc + 1) * 512],
                        in_=ps[:, oc * 512:(oc + 1) * 512])
                    nc.sync.dma_start(
                        out=outp.ap()[sq * 128:(sq + 1) * 128,
                                      oc * 512:(oc + 1) * 512],
                        in_=ost[:, oc * 512:(oc + 1) * 512])
            else:
                for oc in range(2):
                    ps = psum_fl.tile([128, 512], dt.float32, name="flps")
                    for dvt in range(4):
                        nc.tensor.matmul(
                            ps,
                            lhsT=cxtq_sb[(dvt, sq)],
                            rhs=ow_sb[dvt][:, oc * 512:(oc + 1) * 512],
                            start=(dvt == 0),
                            stop=(dvt == 3),
                        )
                    nc.vector.tensor_copy(
                        out=ost[:, oc * 512:(oc + 1) * 512], in_=ps)
                nc.sync.dma_start(
                    out=outp.ap()[sq * 128:(sq + 1) * 128, :], in_=ost)

    UNIT_PE = {"q": 1707, "k": 1707, "v": 1707, "p4": 1707}
    emitted = set()
    lazy_q = []
    # bal tracks the LOCAL PE-idle credit (ACT minus PE time of recent
    # iterations). It is clamped from below: a burst of forced units leaves
    # only a bounded PE backlog because the exp stream itself stalls behind
    # the PE and the two streams re-synchronize.
    clock = {"bal": 0.0}

    def emit_unit(key):
        canon = key[:2] if key[0] == "p4" else key
        if canon in emitted:
            return
        emitted.add(canon)
        emit_unit_fn(key)
        clock["bal"] = max(clock["bal"] - UNIT_PE[key[0]], 0.0)

    def pop_lazy():
        while lazy_q:
            key = lazy_q[0]
            if (key[:2] if key[0] == "p4" else key) in emitted:
                lazy_q.pop(0)
                continue
            if clock["bal"] < UNIT_PE[key[0]]:
                break
            emit_unit(lazy_q.pop(0))

    # ===================== attention block =================================
    def attn_block(qc, hp, fast_drain=False, on_sq_ready=None):
        """512-wide q chunk qc for heads h0=2*hp (PE rows 0:64) and
        h1=2*hp+1 (rows 64:128).

        Two k-blocks share one [128,1024] ST psum tile so each exp covers
        ~1024 cols; the causal mask is a -1e9 accumulating matmul on the
        diagonal blocks; PV (PT as lhsT, [V|1] as rhs, out [128 q, 65])
        trails two iterations so the PE never waits on the exp. Yields
        between k-block-pair iterations so the driver can interleave filler
        units while the ACT engine is the per-iteration bottleneck."""
        nkb = 4 * qc + 4
        q0 = 512 * qc
        ctxs = [psum_ctx.tile([128, 512], dt.float32, name="ctx")
                for _ in range(2)]
        started = [False, False]
        pend2 = []
        done_j = 0  # qsb's fully accumulated (and, if fast_drain, normed)

        def norm_and_transpose(ja, jb):
            """Normalize qsb's [ja, jb) of both halves and transpose out."""
            nqs = {j: nq_pool.tile([128, 128], dt.bfloat16, name="nq")
                   for j in range(ja, jb)}
            for half in range(2):
                ctx_t = ctxs[half]
                dens = ctx_t[:, 0:260].rearrange(
                    "p (j c) -> p c j", c=65)[:, 64, ja:jb]
                recip = small.tile([128, jb - ja], dt.float32, name="recip")
                nc.vector.reciprocal(out=recip, in_=dens)
                for j in range(ja, jb):
                    nc.vector.tensor_scalar_mul(
                        nqs[j][:, half * 64:half * 64 + 64],
                        ctx_t[:, j * 65:j * 65 + 64],
                        recip[:, j - ja:j - ja + 1],
                    )
            for j in range(ja, jb):
                nc.sync.dma_start_transpose(
                    out=cxtq_sb[(hp, 4 * qc + j)], in_=nqs[j])
                if on_sq_ready is not None:
                    on_sq_ready(4 * qc + j)

        def emit_pv(kbs, offs, ns, pts):
            nonlocal done_j
            for half in range(2):
                for (kb, off, n) in zip(kbs, offs, ns):
                    j0 = (512 - n) // 128
                    for j in range(j0, 4):
                        c0 = off + (j - j0) * 128
                        nc.tensor.matmul(
                            ctxs[half][:, j * 65:j * 65 + 65],
                            lhsT=pts[half][:, c0:c0 + 128],
                            rhs=vo_sb[kb][:, 2 * hp + half, :],
                            start=not started[half],
                            stop=(kb == nkb - 1 and j == 3),
                        )
                        started[half] = True
            if fast_drain:
                new_done = max(kb - 4 * qc + 1 for kb in kbs) \
                    if kbs[-1] >= 4 * qc else 0
                if new_done > done_j:
                    norm_and_transpose(done_j, new_done)
                    done_j = new_done

        for kb0 in range(0, nkb, 2):
            kbs = [kb for kb in (kb0, kb0 + 1) if kb < nkb]
            ns = [512 - max(0, kb * 128 - q0) for kb in kbs]
            offs = [0] + [ns[0]] * (len(kbs) - 1)
            pts = []
            for half in range(2):
                p0 = half * 64
                stp = psum_st.tile([128, 1024], dt.float32, name="stp")
                for kb, off, n in zip(kbs, offs, ns):
                    nc.tensor.matmul(
                        stp[:, off:off + n],
                        lhsT=kt_sb[hp][p0:p0 + 64, kb * 128:(kb + 1) * 128],
                        rhs=qt_sb[hp][p0:p0 + 64, q0 + 512 - n:q0 + 512],
                        start=True,
                        stop=True,
                        tile_position=(p0, 0) if PACK_HEADS else None,
                    )
                ntot = offs[-1] + ns[-1]
                pt = pt_pool.tile([128, 1024], dt.bfloat16, name="pt")
                nc.scalar.activation(
                    out=pt[:, :ntot], in_=stp[:, :ntot], func=F.Exp,
                    scale=0.125)
                for kb, off in zip(kbs, offs):
                    if kb >= 4 * qc:  # diagonal: mask first 128 cols
                        nc.vector.tensor_mul(
                            pt[:, off:off + 128], pt[:, off:off + 128], tri)
                pts.append(pt)
            pend2.append((kbs, offs, ns, pts))
            if len(pend2) > (1 if fast_drain else 4):
                emit_pv(*pend2.pop(0))
            ntot = offs[-1] + ns[-1]
            nqsb = sum(4 - (512 - n) // 128 for n in ns)
            clock["bal"] += 2 * (ntot * 0.8333 + 190) \
                - (2 * ntot + 65 * 2 * nqsb) * 0.4167
            yield
        for p in pend2:
            emit_pv(*p)
        if done_j < 4:
            norm_and_transpose(done_j, 4)

    # ===================== driver ==========================================
    # Anti-diagonal wavefront over (head-pair, q-chunk): each head-pair's
    # q-chunks still run in order (kt accumulates per chunk), but head-pairs
    # are staggered so sq groups finish progressively and the out-proj fills
    # the late ACT-bound windows instead of piling into a tail. QK/V units
    # for later blocks fill the PE between attention iterations, paced
    # against a simple ACT-vs-PE clock; deadline units are forced.
    ORDER = [(0, 0), (0, 1), (1, 0), (1, 1), (0, 2), (2, 0), (0, 3), (1, 2),
             (2, 1), (3, 0), (1, 3), (2, 2), (3, 1), (2, 3), (3, 2), (3, 3)]
    vseen = set()
    for hp, qc in ORDER:
        lazy_q.append(("q", hp, qc))
        lazy_q.append(("k", hp, qc))
        if qc not in vseen:
            vseen.add(qc)
            lazy_q.extend(("v", st) for st in range(4 * qc, 4 * qc + 4))

    def last_sq_ready(sq):
        # the final block: out-proj of each finished sq inline (the filler
        # psum pool is free by then).
        emit_unit(("p4", sq, 0))

    def emit_first_qk():
        """First q/k units with din-halves interleaved to track the
        half-tensor input DMAs, so the PE streams as data arrives."""
        psq = psum_fl.tile([128, 512], dt.float32, name="flps")
        psk = psum_fl.tile([128, 512], dt.float32, name="flps")
        for h in range(4):
            for kind, ps in (("q", psq), ("k", psk)):
                for din in range(2 * h, 2 * h + 2):
                    nc.tensor.matmul(
                        ps,
                        lhsT=W_TILES[kind][din][:, 0:128],
                        rhs=xt_cols(din, 0, 512),
                        start=(din == 0),
                        stop=(din == 7),
                    )
        nc.vector.tensor_copy(out=qt_sb[0][:, 0:512], in_=psq)
        nc.vector.tensor_copy(out=kt_sb[0][:, 0:512], in_=psk)
        emitted.add(("q", 0, 0))
        emitted.add(("k", 0, 0))
        clock["bal"] = max(clock["bal"] - 2 * UNIT_PE["q"], -3000.0)

    emit_first_qk()
    for hp, qc in ORDER:
        last = (hp, qc) == ORDER[-1]
        if last:
            # final block: nothing follows — drain the lazy queue into its
            # ACT-bound iterations instead of a post-exp tail
            clock["bal"] += 2500.0
        emit_unit(("q", hp, qc))
        emit_unit(("k", hp, qc))
        it = attn_block(qc, hp, fast_drain=last,
                        on_sq_ready=last_sq_ready if last else None)
        for i, _ in enumerate(it):
            if i == 0:
                # vo tiles are only read by the PV matmuls, which trail the
                # scores by 2 iterations — forcing V here keeps the first
                # scores off the wv-DMA critical path.
                for st in range(4 * qc, 4 * qc + 4):
                    emit_unit(("v", st))
            pop_lazy()
        if hp == 3:  # sq group qc now has all head-pairs' ctx
            for sq in range(4 * qc, 4 * qc + 4):
                lazy_q.append(("p4", sq, 0))
    # tail: alternate the wide score psum and the filler pool so three
    # psum pairs pipeline the drain copies
    for i, sq in enumerate(range(NKB)):
        emit_unit(("p4", sq, 1 if i % 2 == 0 else 0))

    return pools


def _build_nc():
    import concourse.tile as tile
    from concourse import bacc, mybir

    dt = mybir.dt
    nc = bacc.Bacc("TRN2", target_bir_lowering=False, debug=False,
                   num_devices=NCORES)
    xT = nc.dram_tensor("xt", [D, S], dt.bfloat16, kind="ExternalInput")
    wq = nc.dram_tensor("wq", [D, GD], dt.bfloat16, kind="ExternalInput")
    wk = nc.dram_tensor("wk", [D, GD], dt.bfloat16, kind="ExternalInput")
    wv = nc.dram_tensor("wv", [D, GD], dt.bfloat16, kind="ExternalInput")
    ow = nc.dram_tensor("ow", [GD, D], dt.bfloat16, kind="ExternalInput")
    outp = nc.dram_tensor("outp", [S, D], dt.bfloat16, kind="ExternalOutput")

    with tile.TileContext(nc) as tc:
        pools = _build_body(tc, nc, mybir, xT, wq, wk, wv, ow, outp)
        pools.close()
    nc.compile()
    return nc


LAST_RESULTS = None


def kernel(batch, w_query, w_key, w_value, out_w, out_b):
    global LAST_RESULTS
    import os
    from concourse import bass_utils

    try:  # BASS_TRACE needs the axon NTFF hook; without it the run crashes
        from antenv.axon_hooks import get_axon_ntff_profile_hook  # noqa: F401
    except ImportError:
        os.environ.setdefault("BASS_NEVER_TRACE", "1")

    batch = np.asarray(batch, dtype=np.float32)
    w_query = np.asarray(w_query, dtype=np.float32)
    w_key = np.asarray(w_key, dtype=np.float32)
    w_value = np.asarray(w_value, dtype=np.float32)
    out_w = np.asarray(out_w, dtype=np.float32)
    out_b = np.asarray(out_b, dtype=np.float32)

    if "nc" not in _cache:
        _cache["nc"] = _build_nc()
    nc = _cache["nc"]

    xts = [np.ascontiguousarray(batch[b].T).astype(BF16) for b in range(B)]
    slc = [slice(g * GD, (g + 1) * GD) for g in range(2)]
    wqs = [np.ascontiguousarray(w_query[:, s]).astype(BF16) for s in slc]
    wks = [np.ascontiguousarray(w_key[:, s]).astype(BF16) for s in slc]
    wvs = [np.ascontiguousarray(w_value[:, s]).astype(BF16) for s in slc]
    ows = [np.ascontiguousarray(out_w[s, :]).astype(BF16) for s in slc]
    in_maps = []
    for c in range(NCORES):
        b, g = divmod(c, 2)
        in_maps.append({
            "xt": xts[b], "wq": wqs[g], "wk": wks[g],
            "wv": wvs[g], "ow": ows[g],
        })

    res = bass_utils.run_bass_kernel_spmd(
        nc, in_maps, core_ids=list(range(NCORES)),
    )
    LAST_RESULTS = res

    out = np.empty((B, S, D), np.float32)
    for b in range(B):
        out[b] = (res.results[2 * b]["outp"].astype(np.float32)
                  + res.results[2 * b + 1]["outp"].astype(np.float32)
                  + out_b[None, :])
    return out


# revision 76
# speedup vs baseline: 1.0053x; 1.0009x over previous
"""Multi-head causal attention (B=4, S=2048, D=1024, 16 heads) on 8 TRN2 cores.

Sharding: core c -> (batch b = c//2, head-group g = c%2). Each core computes
8 heads of one batch element end-to-end (QKV proj, causal softmax attention,
out-proj rows for its head slice). Host sums the two head-group partials per
batch and adds the output bias.

Per-core pipeline (all matmuls contraction-on-partitions, bf16 in / f32 psum):
  attention block per (512-wide q-chunk, head-pair): the pair's score
  matmuls are packed into PE row groups via tile_position; two k-blocks
  share one [128,1024] score psum so each exp covers ~1024 cols; PV matmuls
  trail 4 iterations behind so the PE never waits on the exp:
    ST[k,q] = KT.T @ QT; PT = exp(ST/8) bf16; tri-mask on diagonal 128 cols
    ctx[q 128, h, 65] += PT.T @ [V_h | 1]  (PT slice is the lhsT: output has
      q on partitions -> full 128-row PE output, ~half the PV stream cols of
      the [65, q] orientation; col 64 = softmax denominator)
    nq[q, d] = ctx * (1/denominator)  per-partition scale on DVE
    cxtq[d, q] = DMA-xbar-transpose(nq)  per (head-pair, 128-q tile)
    out row [128, 1024] = cxtq.T @ ow -> bf16 -> DRAM (host upcasts + sums)

Schedule: the exp stream (ACT, ~147us) is the per-iteration bottleneck of a
block (2 exps ~ 2us vs ~1.3us PE), while the QKV projections / out-proj are
pure PE work (~110us). Blocks run in an anti-diagonal (head-pair, q-chunk)
wavefront and every projection / V-tile / out-proj row is a small "filler
unit" interleaved between attention iterations, paced by an ACT-minus-PE
credit so the PE stays ~92% busy end to end. Input DMAs are ordered and
split by first consumption (first exp at ~7us); the final block drains
per-q-subblock so its out-proj overlaps the last exps.
"""

import numpy as np
import ml_dtypes

B, S, D = 4, 2048, 1024
H_TOT = 16
HD = 64
NCORES = 8
GH = 8          # heads per core
GD = GH * HD    # 512: dout slice per core
NKB = S // 128  # 16 k-blocks
NQC = S // 512  # 4 q-chunks
BF16 = ml_dtypes.bfloat16

PACK_HEADS = True   # pack 2 heads' score matmuls into PE row groups

_cache = {}


def _build_body(tc, nc, mybir, xT, wq, wk, wv, ow, outp):
    from concourse.masks import make_upper_triangular
    import contextlib

    dt = mybir.dt
    F = mybir.ActivationFunctionType

    pools = contextlib.ExitStack()
    tc_pool = lambda **kw: pools.enter_context(tc.tile_pool(**kw))

    singles = tc_pool(name="singles", bufs=1)
    pt_pool = tc_pool(name="pt", bufs=12)
    small = tc_pool(name="small", bufs=4)
    nq_pool = tc_pool(name="nq", bufs=12)
    ost_pool = tc_pool(name="ost", bufs=5)
    psum_st = tc_pool(name="psum_st", bufs=2, space="PSUM")
    psum_ctx = tc_pool(name="psum_ctx", bufs=2, space="PSUM")
    psum_fl = tc_pool(name="psum_fl", bufs=2, space="PSUM")

    # ---- persistent SBUF tensors. Inputs live in few merged tiles so each
    # loads with 1-2 large DMAs (HWDGE issue slots are ~625ns each); xT's
    # first 512 q/k columns get their own tile so the first projection units
    # depend on a single early DMA. ----
    xt_c0 = singles.tile([128, 8, 512], dt.bfloat16, name="xtc0")
    xt_r = singles.tile([128, 8, 3 * 512], dt.bfloat16, name="xtr")
    wq_all = singles.tile([128, 8, GD], dt.bfloat16, name="wqa")
    wk_all = singles.tile([128, 8, GD], dt.bfloat16, name="wka")
    wv_all = singles.tile([128, 8, GD], dt.bfloat16, name="wva")
    ow_all = singles.tile([128, 4, D], dt.bfloat16, name="owa")
    wq_sb = [wq_all[:, t, :] for t in range(8)]
    wk_sb = [wk_all[:, t, :] for t in range(8)]
    wv_sb = [wv_all[:, t, :] for t in range(8)]
    ow_sb = [ow_all[:, t, :] for t in range(4)]

    def xt_cols(din, lo, hi):
        """xT tile row `din`, columns [lo, hi) across the c0/rest split."""
        assert lo >= 512 or hi <= 512
        if hi <= 512:
            return xt_c0[:, din, lo:hi]
        return xt_r[:, din, lo - 512:hi - 512]
    qt_sb = [singles.tile([128, S], dt.bfloat16, name=f"qt{t}")
             for t in range(4)]                              # 2 heads / dtile
    kt_sb = [singles.tile([128, S], dt.bfloat16, name=f"kt{t}")
             for t in range(4)]
    vo_sb = [singles.tile([128, GH, 65], dt.bfloat16, name=f"vo{t}")
             for t in range(NKB)]                            # [V_h | ones]
    # normalized ctx^T tiles [128 = 2 heads x 64 d, 128 q] per (pair, seq128)
    cxtq_sb = {(hp, sq): singles.tile([128, 128], dt.bfloat16,
                                      name=f"cq{hp}_{sq}")
               for hp in range(4) for sq in range(NKB)}
    tri = singles.tile([128, 128], dt.bfloat16)              # keep k<=q

    make_upper_triangular(nc, tri, val=1.0, diag=True)
    for t in range(NKB):
        nc.vector.memset(vo_sb[t][:, :, 64:65], 1.0)

    # ---- input DMAs, ordered by first consumption, in half-tensor chunks
    # so the first projection units' low-din matmuls start ~4.5us in. ----
    xT_r = xT.ap().rearrange("(t p) s -> p t s", p=128)
    wq_r = wq.ap().rearrange("(t p) n -> p t n", p=128)
    wk_r = wk.ap().rearrange("(t p) n -> p t n", p=128)
    wv_r = wv.ap().rearrange("(t p) n -> p t n", p=128)
    ow_r = ow.ap().rearrange("(t p) n -> p t n", p=128)
    # head-pair-0 weight columns + the first 512 x columns first: that is
    # the entire critical path to the first exp (~1.5MB).
    nc.sync.dma_start(out=wq_all[:, :, 0:128], in_=wq_r[:, :, 0:128])
    nc.scalar.dma_start(out=wk_all[:, :, 0:128], in_=wk_r[:, :, 0:128])
    for h in range(4):
        ts = slice(2 * h, 2 * h + 2)
        nc.sync.dma_start(out=xt_c0[:, ts, :], in_=xT_r[:, ts, 0:512])
    nc.sync.dma_start(out=wv_all, in_=wv_r)
    nc.sync.dma_start(out=xt_r[:, :, 0:512], in_=xT_r[:, :, 512:1024])
    nc.sync.dma_start(out=xt_r[:, :, 512:1536], in_=xT_r[:, :, 1024:2048])
    nc.sync.dma_start(out=wq_all[:, :, 128:512], in_=wq_r[:, :, 128:512])
    nc.sync.dma_start(out=wk_all[:, :, 128:512], in_=wk_r[:, :, 128:512])
    nc.sync.dma_start(out=ow_all, in_=ow_r)

    # ================= filler units (PE work with no ACT dependency) ======
    # Each unit is a single [128, 512]-output psum bank accumulated over its
    # full contraction + one DVE drain copy. Units are interleaved between
    # attention iterations so the PE stays busy while the ACT engine (the
    # per-iteration bottleneck: 2 exps ~ 2us vs ~1.3us of PE work) churns.
    W_TILES = {"q": wq_sb, "k": wk_sb}
    T_TILES = {"q": qt_sb, "k": kt_sb}

    def emit_unit_fn(key):
        kind = key[0]
        if kind in ("q", "k"):
            _, hp, c = key
            ps = psum_fl.tile([128, 512], dt.float32, name="flps")
            for din in range(8):
                nc.tensor.matmul(
                    ps,
                    lhsT=W_TILES[kind][din][:, hp * 128:(hp + 1) * 128],
                    rhs=xt_cols(din, c * 512, (c + 1) * 512),
                    start=(din == 0),
                    stop=(din == 7),
                )
            nc.vector.tensor_copy(
                out=T_TILES[kind][hp][:, c * 512:(c + 1) * 512], in_=ps)
        elif kind == "v":
            _, st = key
            ps = psum_fl.tile([128, 512], dt.float32, name="flps")
            for din in range(8):
                nc.tensor.matmul(
                    ps,
                    lhsT=xt_cols(din, st * 128, st * 128 + 128),
                    rhs=wv_sb[din],
                    start=(din == 0),
                    stop=(din == 7),
                )
            nc.vector.tensor_copy(
                out=vo_sb[st][:, :, 0:64],
                in_=ps.rearrange("p (h d) -> p h d", h=GH),
            )
        else:  # ("p4", sq, mode): full [128, 1024] out row in bf16
            _, sq, mode = key
            ost = ost_pool.tile([128, 1024], dt.bfloat16, name="ost")
            if mode == 1:  # tail: scores done, use the wide score psum;
                # half copies + half DMAs so the write-out starts early
                ps = psum_st.tile([128, 1024], dt.float32, name="stp")
                for oc in range(2):
                    for dvt in range(4):
                        nc.tensor.matmul(
                            ps[:, oc * 512:(oc + 1) * 512],
                            lhsT=cxtq_sb[(dvt, sq)],
                            rhs=ow_sb[dvt][:, oc * 512:(oc + 1) * 512],
                            start=(dvt == 0),
                            stop=(dvt == 3),
                        )
                    nc.vector.tensor_copy(
                        out=ost[:, oc * 512:(oc + 1) * 512],
                        in_=ps[:, oc * 512:(oc + 1) * 512])
                    nc.sync.dma_start(
                        out=outp.ap()[sq * 128:(sq + 1) * 128,
                                      oc * 512:(oc + 1) * 512],
                        in_=ost[:, oc * 512:(oc + 1) * 512])
            else:
                for oc in range(2):
                    ps = psum_fl.tile([128, 512], dt.float32, name="flps")
                    for dvt in range(4):
                        nc.tensor.matmul(
                            ps,
                            lhsT=cxtq_sb[(dvt, sq)],
                            rhs=ow_sb[dvt][:, oc * 512:(oc + 1) * 512],
                            start=(dvt == 0),
                            stop=(dvt == 3),
                        )
                    nc.vector.tensor_copy(
                        out=ost[:, oc * 512:(oc + 1) * 512], in_=ps)
                nc.sync.dma_start(
                    out=outp.ap()[sq * 128:(sq + 1) * 128, :], in_=ost)

    UNIT_PE = {"q": 1707, "k": 1707, "v": 1707, "p4": 1707}
    emitted = set()
    lazy_q = []
    # bal tracks the LOCAL PE-idle credit (ACT minus PE time of recent
    # iterations). It is clamped from below: a burst of forced units leaves
    # only a bounded PE backlog because the exp stream itself stalls behind
    # the PE and the two streams re-synchronize.
    clock = {"bal": 0.0}

    def emit_unit(key):
        canon = key[:2] if key[0] == "p4" else key
        if canon in emitted:
            return
        emitted.add(canon)
        emit_unit_fn(key)
        clock["bal"] = max(clock["bal"] - UNIT_PE[key[0]], 0.0)

    def pop_lazy():
        while lazy_q:
            key = lazy_q[0]
            if (key[:2] if key[0] == "p4" else key) in emitted:
                lazy_q.pop(0)
                continue
            if clock["bal"] < UNIT_PE[key[0]]:
                break
            emit_unit(lazy_q.pop(0))

    # ===================== attention block =================================
    def attn_block(qc, hp, fast_drain=False, on_sq_ready=None):
        """512-wide q chunk qc for heads h0=2*hp (PE rows 0:64) and
        h1=2*hp+1 (rows 64:128).

        Two k-blocks share one [128,1024] ST psum tile so each exp covers
        ~1024 cols; the causal mask is a -1e9 accumulating matmul on the
        diagonal blocks; PV (PT as lhsT, [V|1] as rhs, out [128 q, 65])
        trails two iterations so the PE never waits on the exp. Yields
        between k-block-pair iterations so the driver can interleave filler
        units while the ACT engine is the per-iteration bottleneck."""
        nkb = 4 * qc + 4
        q0 = 512 * qc
        ctxs = [psum_ctx.tile([128, 512], dt.float32, name="ctx")
                for _ in range(2)]
        started = [False, False]
        pend2 = []
        done_j = 0  # qsb's fully accumulated (and, if fast_drain, normed)

        def norm_and_transpose(ja, jb):
            """Normalize qsb's [ja, jb) of both halves and transpose out."""
            nqs = {j: nq_pool.tile([128, 128], dt.bfloat16, name="nq")
                   for j in range(ja, jb)}
            for half in range(2):
                ctx_t = ctxs[half]
                dens = ctx_t[:, 0:260].rearrange(
                    "p (j c) -> p c j", c=65)[:, 64, ja:jb]
                recip = small.tile([128, jb - ja], dt.float32, name="recip")
                nc.vector.reciprocal(out=recip, in_=dens)
                for j in range(ja, jb):
                    nc.vector.tensor_scalar_mul(
                        nqs[j][:, half * 64:half * 64 + 64],
                        ctx_t[:, j * 65:j * 65 + 64],
                        recip[:, j - ja:j - ja + 1],
                    )
            for j in range(ja, jb):
                nc.sync.dma_start_transpose(
                    out=cxtq_sb[(hp, 4 * qc + j)], in_=nqs[j])
                if on_sq_ready is not None:
                    on_sq_ready(4 * qc + j)

        def emit_pv(kbs, offs, ns, pts):
            nonlocal done_j
            for half in range(2):
                for (kb, off, n) in zip(kbs, offs, ns):
                    j0 = (512 - n) // 128
                    for j in range(j0, 4):
                        c0 = off + (j - j0) * 128
                        nc.tensor.matmul(
                            ctxs[half][:, j * 65:j * 65 + 65],
                            lhsT=pts[half][:, c0:c0 + 128],
                            rhs=vo_sb[kb][:, 2 * hp + half, :],
                            start=not started[half],
                            stop=(kb == nkb - 1 and j == 3),
                        )
                        started[half] = True
            if fast_drain:
                new_done = max(kb - 4 * qc + 1 for kb in kbs) \
                    if kbs[-1] >= 4 * qc else 0
                if new_done > done_j:
                    norm_and_transpose(done_j, new_done)
                    done_j = new_done

        for kb0 in range(0, nkb, 2):
            kbs = [kb for kb in (kb0, kb0 + 1) if kb < nkb]
            ns = [512 - max(0, kb * 128 - q0) for kb in kbs]
            offs = [0] + [ns[0]] * (len(kbs) - 1)
            pts = []
            for half in range(2):
                p0 = half * 64
                stp = psum_st.tile([128, 1024], dt.float32, name="stp")
                for kb, off, n in zip(kbs, offs, ns):
                    nc.tensor.matmul(
                        stp[:, off:off + n],
                        lhsT=kt_sb[hp][p0:p0 + 64, kb * 128:(kb + 1) * 128],
                        rhs=qt_sb[hp][p0:p0 + 64, q0 + 512 - n:q0 + 512],
                        start=True,
                        stop=True,
                        tile_position=(p0, 0) if PACK_HEADS else None,
                    )
                ntot = offs[-1] + ns[-1]
                pt = pt_pool.tile([128, 1024], dt.bfloat16, name="pt")
                nc.scalar.activation(
                    out=pt[:, :ntot], in_=stp[:, :ntot], func=F.Exp,
                    scale=0.125)
                for kb, off in zip(kbs, offs):
                    if kb >= 4 * qc:  # diagonal: mask first 128 cols
                        nc.vector.tensor_mul(
                            pt[:, off:off + 128], pt[:, off:off + 128], tri)
                pts.append(pt)
            pend2.append((kbs, offs, ns, pts))
            if len(pend2) > (1 if fast_drain else 5):
                emit_pv(*pend2.pop(0))
            ntot = offs[-1] + ns[-1]
            nqsb = sum(4 - (512 - n) // 128 for n in ns)
            clock["bal"] += 2 * (ntot * 0.8333 + 190) \
                - (2 * ntot + 65 * 2 * nqsb) * 0.4167
            yield
        for p in pend2:
            emit_pv(*p)
        if done_j < 4:
            norm_and_transpose(done_j, 4)

    # ===================== driver ==========================================
    # Anti-diagonal wavefront over (head-pair, q-chunk): each head-pair's
    # q-chunks still run in order (kt accumulates per chunk), but head-pairs
    # are staggered so sq groups finish progressively and the out-proj fills
    # the late ACT-bound windows instead of piling into a tail. QK/V units
    # for later blocks fill the PE between attention iterations, paced
    # against a simple ACT-vs-PE clock; deadline units are forced.
    ORDER = [(0, 0), (0, 1), (1, 0), (1, 1), (0, 2), (2, 0), (0, 3), (1, 2),
             (2, 1), (3, 0), (1, 3), (2, 2), (3, 1), (2, 3), (3, 2), (3, 3)]
    vseen = set()
    for hp, qc in ORDER:
        lazy_q.append(("q", hp, qc))
        lazy_q.append(("k", hp, qc))
        if qc not in vseen:
            vseen.add(qc)
            lazy_q.extend(("v", st) for st in range(4 * qc, 4 * qc + 4))

    def last_sq_ready(sq):
        # the final block: out-proj of each finished sq inline (the filler
        # psum pool is free by then).
        emit_unit(("p4", sq, 0))

    def emit_first_qk():
        """First q/k units with din-halves interleaved to track the
        half-tensor input DMAs, so the PE streams as data arrives."""
        psq = psum_fl.tile([128, 512], dt.float32, name="flps")
        psk = psum_fl.tile([128, 512], dt.float32, name="flps")
        for h in range(4):
            for kind, ps in (("q", psq), ("k", psk)):
                for din in range(2 * h, 2 * h + 2):
                    nc.tensor.matmul(
                        ps,
                        lhsT=W_TILES[kind][din][:, 0:128],
                        rhs=xt_cols(din, 0, 512),
                        start=(din == 0),
                        stop=(din == 7),
                    )
        nc.vector.tensor_copy(out=qt_sb[0][:, 0:512], in_=psq)
        nc.vector.tensor_copy(out=kt_sb[0][:, 0:512], in_=psk)
        emitted.add(("q", 0, 0))
        emitted.add(("k", 0, 0))
        clock["bal"] = max(clock["bal"] - 2 * UNIT_PE["q"], -3000.0)

    emit_first_qk()
    for hp, qc in ORDER:
        last = (hp, qc) == ORDER[-1]
        if last:
            # final block: nothing follows — drain the lazy queue into its
            # ACT-bound iterations instead of a post-exp tail
            clock["bal"] += 2500.0
        emit_unit(("q", hp, qc))
        emit_unit(("k", hp, qc))
        it = attn_block(qc, hp, fast_drain=last,
                        on_sq_ready=last_sq_ready if last else None)
        for i, _ in enumerate(it):
            if i == 0:
                # vo tiles are only read by the PV matmuls, which trail the
                # scores by 2 iterations — forcing V here keeps the first
                # scores off the wv-DMA critical path.
                for st in range(4 * qc, 4 * qc + 4):
                    emit_unit(("v", st))
            pop_lazy()
        if hp == 3:  # sq group qc now has all head-pairs' ctx
            for sq in range(4 * qc, 4 * qc + 4):
                lazy_q.append(("p4", sq, 0))
    # tail: alternate the wide score psum and the filler pool so three
    # psum pairs pipeline the drain copies
    for i, sq in enumerate(range(NKB)):
        emit_unit(("p4", sq, 1 if i % 2 == 0 else 0))

    return pools


def _build_nc():
    import concourse.tile as tile
    from concourse import bacc, mybir

    dt = mybir.dt
    nc = bacc.Bacc("TRN2", target_bir_lowering=False, debug=False,
                   num_devices=NCORES)
    xT = nc.dram_tensor("xt", [D, S], dt.bfloat16, kind="ExternalInput")
    wq = nc.dram_tensor("wq", [D, GD], dt.bfloat16, kind="ExternalInput")
    wk = nc.dram_tensor("wk", [D, GD], dt.bfloat16, kind="ExternalInput")
    wv = nc.dram_tensor("wv", [D, GD], dt.bfloat16, kind="ExternalInput")
    ow = nc.dram_tensor("ow", [GD, D], dt.bfloat16, kind="ExternalInput")
    outp = nc.dram_tensor("outp", [S, D], dt.bfloat16, kind="ExternalOutput")

    with tile.TileContext(nc) as tc:
        pools = _build_body(tc, nc, mybir, xT, wq, wk, wv, ow, outp)
        pools.close()
    nc.compile()
    return nc


LAST_RESULTS = None


def kernel(batch, w_query, w_key, w_value, out_w, out_b):
    global LAST_RESULTS
    import os
    from concourse import bass_utils

    try:  # BASS_TRACE needs the axon NTFF hook; without it the run crashes
        from antenv.axon_hooks import get_axon_ntff_profile_hook  # noqa: F401
    except ImportError:
        os.environ.setdefault("BASS_NEVER_TRACE", "1")

    batch = np.asarray(batch, dtype=np.float32)
    w_query = np.asarray(w_query, dtype=np.float32)
    w_key = np.asarray(w_key, dtype=np.float32)
    w_value = np.asarray(w_value, dtype=np.float32)
    out_w = np.asarray(out_w, dtype=np.float32)
    out_b = np.asarray(out_b, dtype=np.float32)

    if "nc" not in _cache:
        _cache["nc"] = _build_nc()
    nc = _cache["nc"]

    xts = [np.ascontiguousarray(batch[b].T).astype(BF16) for b in range(B)]
    slc = [slice(g * GD, (g + 1) * GD) for g in range(2)]
    wqs = [np.ascontiguousarray(w_query[:, s]).astype(BF16) for s in slc]
    wks = [np.ascontiguousarray(w_key[:, s]).astype(BF16) for s in slc]
    wvs = [np.ascontiguousarray(w_value[:, s]).astype(BF16) for s in slc]
    ows = [np.ascontiguousarray(out_w[s, :]).astype(BF16) for s in slc]
    in_maps = []
    for c in range(NCORES):
        b, g = divmod(c, 2)
        in_maps.append({
            "xt": xts[b], "wq": wqs[g], "wk": wks[g],
            "wv": wvs[g], "ow": ows[g],
        })

    res = bass_utils.run_bass_kernel_spmd(
        nc, in_maps, core_ids=list(range(NCORES)),
    )
    LAST_RESULTS = res

    out = np.empty((B, S, D), np.float32)
    for b in range(B):
        out[b] = (res.results[2 * b]["outp"].astype(np.float32)
                  + res.results[2 * b + 1]["outp"].astype(np.float32)
                  + out_b[None, :])
    return out
